# revision 13
# baseline (speedup 1.0000x reference)
"""Trainium2 Bass kernel for the BigSolDB pretrain model (two GIN encoders +
fusion head), distributed over 8 NeuronCores.

Sharding: cores 0-3 run the drug encoder, cores 4-7 the solvent encoder
(identical SPMD program, different per-core data).  Within each group of 4,
nodes are sharded into 4 contiguous ranges; each core handles the edges whose
destination falls in its range.  Per GIN layer the full node-feature table is
rebuilt with an AllGather so every core can gather arbitrary source rows, and
batch-norm statistics are combined with a small AllReduce.  Pooled graph
embeddings are exchanged across the two groups and the fusion head is computed
redundantly on every core.

The edge aggregation (segment_sum of x[src] by dst) runs as:
  - batched indirect DMA row gathers from the HBM feature table, and
  - PE matmuls against on-chip-built one-hot (edge -> dst slot) matrices,
    accumulating each 128-wide dst window in PSUM.
"""

import numpy as np

import concourse.bass as bass
import concourse.bacc as bacc
import concourse.mybir as mybir
import concourse.tile as tile

F32 = mybir.dt.float32
I32 = mybir.dt.int32
AF = mybir.ActivationFunctionType
ALU = mybir.AluOpType

HID = 128
HID2 = 256
NCORES = 8
GS = 4          # cores per encoder group
WIN = 128       # dst-window width (one-hot matmul N dim)
CH = 512        # node-chunk width for the MLP phases
BN_EPS = 1e-5


# ---------------------------------------------------------------------------
# Host-side sharding / scheduling
# ---------------------------------------------------------------------------

def _shard_edges(edge_index, n_nodes, nsh):
    """Split edges by dst shard; per shard return (src, dst_local) sorted by
    dst_local."""
    src = np.asarray(edge_index[0], dtype=np.int64)
    dst = np.asarray(edge_index[1], dtype=np.int64)
    shards = []
    for s in range(GS):
        lo, hi = s * nsh, (s + 1) * nsh
        m = (dst >= lo) & (dst < hi)
        ss, dd = src[m], dst[m] - lo
        order = np.argsort(dd, kind="stable")
        shards.append((ss[order], dd[order]))
    return shards


def _window_counts(dst_local, nsh):
    nw = -(-nsh // WIN)
    return np.bincount(dst_local // WIN, minlength=nw)


def _window_stream_counts(src, dst_local, nsh, split):
    """Per window: (lo_count, hi_count) by src < split."""
    nw = -(-nsh // WIN)
    w_of_edge = dst_local // WIN
    lo = src < split
    clo = np.bincount(w_of_edge[lo], minlength=nw)
    chi = np.bincount(w_of_edge[~lo], minlength=nw)
    return clo, chi


def _build_edge_arrays(src, dst_local, nsh, T_lo, T_hi, split):
    """Pack edges into the shared (window, lo-tiles, hi-tiles) grid.

    Returns idx16 [128, NT*8] int16 (dma_gather wrapped layout, replicated
    into the 8 Q7-core stripes) and dst_f [128, NT] float32 (-1 in pad
    slots)."""
    nw = len(T_lo)
    NT = int(np.sum(T_lo) + np.sum(T_hi))
    idx16 = np.zeros((128, NT * 8), dtype=np.int16)
    dst_a = np.full((128, NT), -1.0, dtype=np.float32)
    w_of_edge = dst_local // WIN
    starts = np.searchsorted(w_of_edge, np.arange(nw))
    ends = np.searchsorted(w_of_edge, np.arange(nw) + 1)
    base = 0
    for w in range(nw):
        sl = slice(starts[w], ends[w])
        ws, wd = src[sl], dst_local[sl]
        lo_m = ws < split
        for stream, Ts in ((0, T_lo[w]), (1, T_hi[w])):
            if Ts == 0:
                continue
            m = lo_m if stream == 0 else ~lo_m
            es = ws[m] - (0 if stream == 0 else split)
            ed = wd[m]
            cnt = len(es)
            flat_s = np.zeros(Ts * 128, dtype=np.int16)
            flat_d = np.full(Ts * 128, -1.0, dtype=np.float32)
            flat_s[:cnt] = es.astype(np.int16)
            flat_d[:cnt] = ed - w * WIN
            dst_a[:, base:base + Ts] = flat_d.reshape(Ts, 128).T
            k = np.arange(Ts * 128)
            cols = base * 8 + k // 16
            for c in range(8):
                idx16[16 * c + (k % 16), cols] = flat_s
            base += Ts
    return idx16, dst_a


def _batch_array(batch, nsh, s):
    """[128, NW] float32 graph ids of this shard's nodes, -1 in pad slots."""
    nw = -(-nsh // WIN)
    out = np.full((128, nw), -1.0, dtype=np.float32)
    vals = np.asarray(batch[s * nsh:(s + 1) * nsh], dtype=np.float32)
    pad = nw * WIN - len(vals)
    if pad:
        vals = np.concatenate([vals, np.full(pad, -1.0, np.float32)])
    out[:, :] = vals.reshape(nw, 128).T
    return out


def _chunks(n, ch):
    return [(c0, min(ch, n - c0)) for c0 in range(0, n, ch)]


# ---------------------------------------------------------------------------
# Program builder (one SPMD program shared by all 8 cores)
# ---------------------------------------------------------------------------

def build_program(cfg):
    NSH = cfg["NSH"]
    N = GS * NSH
    NW = -(-NSH // WIN)
    SPLIT = cfg["SPLIT"]
    T_lo = cfg["T_lo"]
    T_hi = cfg["T_hi"]
    Ttot = [T_lo[w] + T_hi[w] for w in range(NW)]
    NT = int(np.sum(Ttot))
    Tmax = max(1, max(Ttot))
    HAS_HI = N > SPLIT
    B = cfg["B"]
    FIN = cfg["FIN"]
    NL = cfg["NL"]
    GH = -(-B // 128)            # graph-halves (1 if B<=128, else 2)
    chunks = _chunks(NSH, CH)
    NCH = len(chunks)
    inv_n = 1.0 / float(N)
    enc_groups = [list(range(GS)), list(range(GS, 2 * GS))]
    pair_groups = [[c, c + GS] for c in range(GS)]

    nc = bacc.Bacc("TRN2", target_bir_lowering=False, debug=False,
                   num_devices=NCORES)

    def din(name, shape, dtype=F32):
        return nc.dram_tensor(name, shape, dtype, kind="ExternalInput")

    xin = din("xin", [FIN, NSH])
    srcs_d = din("srcs", [128, NT * 8], mybir.dt.int16)
    dsts_d = din("dsts", [128, NT])
    batf_d = din("batf", [128, NW])
    io128_d = din("io128", [128, WIN])
    iog_d = din("iog", [128, B])
    idn_d = din("idn", [128, 128])
    ones_d = din("ones", [128, 1])
    trow_d = din("trow", [1, B])

    embw_d = din("emb_w", [FIN, HID])
    embb_d = din("emb_b", [HID, 1])
    lp = []
    for l in range(NL):
        lp.append({k: din(f"{k}_{l}", shp) for k, shp in [
            ("w1", [HID, HID2]),
            ("g1a", [128, 1]), ("g1b", [128, 1]),
            ("be1a", [128, 1]), ("be1b", [128, 1]),
            ("w2a", [128, HID]), ("w2b", [128, HID]),
            ("g2", [HID, 1]), ("be2", [HID, 1]),
            ("epsi", [128, 128]),
        ]})
    hp = {k: din(k, shp) for k, shp in [
        ("pw1a", [128, 128]), ("pw1b", [128, 128]), ("pw1c", [32, 128]),
        ("pb1", [128, 1]), ("pw2", [128, 64]), ("pb2", [64, 1]),
        ("pw3", [64, 1]), ("pb3", [1, 1]),
        ("tw1", [1, 32]), ("tb1", [32, 1]), ("tw2", [32, 32]), ("tb2", [32, 1]),
    ]}

    out_d = nc.dram_tensor("out", [1, B], F32, kind="ExternalOutput")

    # internal DRAM
    xtab_in = nc.dram_tensor("xtab_in", [NSH, HID], F32)
    x_table = nc.dram_tensor("x_table", [N, HID], F32)
    xtab_hi = (nc.dram_tensor("xtab_hi", [N - SPLIT, HID], F32)
               if HAS_HI else None)
    st_bufs = []
    for l in range(NL):
        st_bufs.append((
            nc.dram_tensor(f"s1i_{l}", [128, 4], F32),
            nc.dram_tensor(f"s1o_{l}", [128, 4], F32),
            nc.dram_tensor(f"s2i_{l}", [128, 2], F32),
            nc.dram_tensor(f"s2o_{l}", [128, 2], F32),
        ))
    GH_ = -(-B // 128)
    pacc_in = nc.dram_tensor("pacc_in", [128, GH_ * 132], F32)
    pacc_out = nc.dram_tensor("pacc_out", [128, GH_ * 132], F32)
    pool_in = nc.dram_tensor("pool_in", [128, B], F32)
    pool_out = nc.dram_tensor("pool_out", [256, B], F32)

    with tile.TileContext(nc) as tc:
        with (
            tc.tile_pool(name="persist", bufs=1) as pp,
            tc.tile_pool(name="wrk", bufs=3) as wk,
            tc.tile_pool(name="gat", bufs=3) as gp,
        ):
            # ---- persistent loads ----
            xT = pp.tile([128, NSH], F32, tag="xT")
            dsts = pp.tile([128, NT], F32, tag="dsts")
            batf = pp.tile([128, NW], F32, tag="batf")
            io128 = pp.tile([128, WIN], F32, tag="io128")
            iog = pp.tile([128, B], F32, tag="iog")
            idn = pp.tile([128, 128], F32, tag="idn")
            onec = pp.tile([128, 1], F32, tag="onec")
            trow = pp.tile([1, B], F32, tag="trow")
            ssum = pp.tile([128, 2 * NCH], F32, tag="ssum")
            ssq = pp.tile([128, 2 * NCH], F32, tag="ssq")
            s2sum = pp.tile([128, NCH], F32, tag="s2sum")
            s2sq = pp.tile([128, NCH], F32, tag="s2sq")

            nc.sync.dma_start(out=dsts[:], in_=dsts_d[:, :])
            nc.sync.dma_start(out=batf[:], in_=batf_d[:, :])
            nc.sync.dma_start(out=io128[:], in_=io128_d[:, :])
            nc.sync.dma_start(out=iog[:], in_=iog_d[:, :])
            nc.sync.dma_start(out=idn[:], in_=idn_d[:, :])
            nc.sync.dma_start(out=onec[:], in_=ones_d[:, :])
            nc.sync.dma_start(out=trow[:], in_=trow_d[:, :])

            embw = pp.tile([FIN, HID], F32, tag="embw")
            embb = pp.tile([HID, 1], F32, tag="embb")
            nc.sync.dma_start(out=embw[:], in_=embw_d[:, :])
            nc.sync.dma_start(out=embb[:], in_=embb_d[:, :])
            lt = []
            for l in range(NL):
                d = {}
                for k, h in lp[l].items():
                    d[k] = pp.tile(list(h.shape), F32, tag=f"{k}_{l}",
                                   name=f"{k}_{l}_t")
                    nc.sync.dma_start(out=d[k][:], in_=h[:, :])
                lt.append(d)
            ht = {}
            for k, h in hp.items():
                ht[k] = pp.tile(list(h.shape), F32, tag=k, name=f"{k}_t")
                nc.sync.dma_start(out=ht[k][:], in_=h[:, :])

            with (
                tc.tile_pool(name="pq", bufs=3, space="PSUM") as pqp,
                tc.tile_pool(name="pmlp", bufs=2, space="PSUM") as pmlp,
            ):
                # ---- embedding: xT = emb_w.T @ xin (+ emb_b) ----
                for ci, (c0, cw) in enumerate(chunks):
                    xc = wk.tile([FIN, CH], F32, tag="xc")
                    nc.sync.dma_start(out=xc[:, :cw], in_=xin[:, c0:c0 + cw])
                    pe = pmlp.tile([128, CH], F32, space="PSUM", tag="pm0")
                    nc.tensor.matmul(out=pe[:, :cw], lhsT=embw[:],
                                     rhs=xc[:, :cw], start=True, stop=True)
                    nc.vector.tensor_scalar_add(out=xT[:, c0:c0 + cw],
                                                in0=pe[:, :cw],
                                                scalar1=embb[:, :1])

                # ---- GIN layers ----
                for l in range(NL):
                    d = lt[l]
                    s1i, s1o, s2i, s2o = st_bufs[l]

                    # 1) rebuild the gather table: x_table <- AllGather(x_loc)
                    for w in range(NW):
                        w0 = w * WIN
                        ww = min(WIN, NSH - w0)
                        tp = pqp.tile([128, 128], F32, space="PSUM", tag="pq")
                        nc.tensor.transpose(out=tp[:ww, :],
                                            in_=xT[:, w0:w0 + ww],
                                            identity=idn[:])
                        ts = wk.tile([128, 128], F32, tag="tts")
                        nc.scalar.copy(out=ts[:ww, :], in_=tp[:ww, :])
                        nc.sync.dma_start(out=xtab_in[w0:w0 + ww, :],
                                          in_=ts[:ww, :])
                    nc.gpsimd.collective_compute(
                        "AllGather", ALU.bypass, replica_groups=enc_groups,
                        ins=[xtab_in.ap().opt()], outs=[x_table.ap().opt()])
                    if HAS_HI:
                        nc.sync.dma_start(out=xtab_hi[:, :],
                                          in_=x_table[SPLIT:, :])

                    # 2) aggregate + h = (1+eps)*x + agg   (in place on xT)
                    base = 0
                    for w in range(NW):
                        Tl, Th = T_lo[w], T_hi[w]
                        Tw = Tl + Th
                        w0 = w * WIN
                        ww = min(WIN, NSH - w0)
                        pag = pqp.tile([128, WIN], F32, space="PSUM", tag="pq")
                        nc.tensor.matmul(out=pag[:, :ww], lhsT=d["epsi"][:],
                                         rhs=xT[:, w0:w0 + ww],
                                         start=True, stop=(Tw == 0))
                        if Tw > 0:
                            idxw = gp.tile([128, Tmax * 8], mybir.dt.int16,
                                           tag="idxw")
                            nc.sync.dma_start(
                                out=idxw[:, :Tw * 8],
                                in_=srcs_d[:, base * 8:(base + Tw) * 8])
                            gt = gp.tile([128, Tmax * HID], F32, tag="gt")
                            if Tl > 0:
                                nc.gpsimd.dma_gather(
                                    out_ap=gt[:, :Tl * HID].rearrange(
                                        "p (t d) -> p t d", d=HID),
                                    in_ap=x_table[:, :],
                                    idxs_ap=idxw[:, :Tl * 8],
                                    num_idxs=Tl * 128,
                                    num_idxs_reg=Tl * 128,
                                    elem_size=HID,
                                    single_packet=False)
                            if Th > 0:
                                nc.gpsimd.dma_gather(
                                    out_ap=gt[:, Tl * 8 * HID // 8:Tw * HID].rearrange(
                                        "p (t d) -> p t d", d=HID),
                                    in_ap=xtab_hi[:, :],
                                    idxs_ap=idxw[:, Tl * 8:Tw * 8],
                                    num_idxs=Th * 128,
                                    num_idxs_reg=Th * 128,
                                    elem_size=HID,
                                    single_packet=False)
                            mt = gp.tile([128, Tmax * WIN], F32, tag="mt")
                            din_ap = dsts[:, base:base + Tw]
                            in0 = bass.AP(din_ap.tensor, din_ap.offset,
                                          [din_ap.ap[0], [din_ap.ap[1][0], Tw],
                                           [0, ww]])
                            io_ap = io128[:, :ww]
                            in1 = bass.AP(io_ap.tensor, io_ap.offset,
                                          [io_ap.ap[0], [0, Tw], [1, ww]])
                            nc.vector.tensor_tensor(
                                out=mt[:, :Tw * ww].rearrange(
                                    "p (t w) -> p t w", t=Tw),
                                in0=in0, in1=in1, op=ALU.is_equal)
                            for t in range(Tw):
                                nc.tensor.matmul(
                                    out=pag[:, :ww],
                                    lhsT=gt[:, t * HID:(t + 1) * HID],
                                    rhs=mt[:, t * ww:(t + 1) * ww],
                                    start=False, stop=(t == Tw - 1))
                            base += Tw
                        nc.vector.tensor_copy(out=xT[:, w0:w0 + ww],
                                              in_=pag[:, :ww])

                    # 3) first-pass matmul1 for BN1 statistics
                    for ci, (c0, cw) in enumerate(chunks):
                        for hf in range(2):
                            pm = pmlp.tile([128, CH], F32, space="PSUM",
                                           tag=f"pm{hf}")
                            nc.tensor.matmul(
                                out=pm[:, :cw],
                                lhsT=d["w1"][:, hf * 128:(hf + 1) * 128],
                                rhs=xT[:, c0:c0 + cw], start=True, stop=True)
                            sc1 = wk.tile([128, CH], F32, tag="sc1")
                            nc.scalar.activation(
                                out=sc1[:, :cw], in_=pm[:, :cw], func=AF.Copy,
                                accum_out=ssum[:, hf * NCH + ci:hf * NCH + ci + 1])
                            sc2 = wk.tile([128, CH], F32, tag="sc2")
                            nc.scalar.activation(
                                out=sc2[:, :cw], in_=pm[:, :cw], func=AF.Square,
                                accum_out=ssq[:, hf * NCH + ci:hf * NCH + ci + 1])

                    st = wk.tile([128, 4], F32, tag="st")
                    for hf in range(2):
                        nc.vector.tensor_reduce(
                            out=st[:, hf:hf + 1],
                            in_=ssum[:, hf * NCH:(hf + 1) * NCH],
                            axis=mybir.AxisListType.X, op=ALU.add)
                        nc.vector.tensor_reduce(
                            out=st[:, 2 + hf:3 + hf],
                            in_=ssq[:, hf * NCH:(hf + 1) * NCH],
                            axis=mybir.AxisListType.X, op=ALU.add)
                    nc.sync.dma_start(out=s1i[:, :], in_=st[:])
                    nc.gpsimd.collective_compute(
                        "AllReduce", ALU.add, replica_groups=enc_groups,
                        ins=[s1i.ap().opt()], outs=[s1o.ap().opt()])
                    sr = wk.tile([128, 4], F32, tag="sr")
                    nc.sync.dma_start(out=sr[:], in_=s1o[:, :])

                    sca, sha = [], []
                    for hf in range(2):
                        g_t = d["g1a"] if hf == 0 else d["g1b"]
                        be_t = d["be1a"] if hf == 0 else d["be1b"]
                        mean = wk.tile([128, 1], F32, tag=f"mean{hf}")
                        nc.vector.tensor_scalar_mul(
                            out=mean[:], in0=sr[:, hf:hf + 1], scalar1=inv_n)
                        var = wk.tile([128, 1], F32, tag=f"var{hf}")
                        # var = sq/n - mean^2 ; then + eps
                        nc.vector.tensor_scalar_mul(
                            out=var[:], in0=sr[:, 2 + hf:3 + hf], scalar1=inv_n)
                        m2 = wk.tile([128, 1], F32, tag=f"m2{hf}")
                        nc.vector.tensor_tensor(out=m2[:], in0=mean[:],
                                                in1=mean[:], op=ALU.mult)
                        nc.vector.tensor_tensor(out=var[:], in0=var[:],
                                                in1=m2[:], op=ALU.subtract)
                        nc.vector.tensor_scalar_add(out=var[:], in0=var[:],
                                                    scalar1=BN_EPS)
                        nc.vector.reciprocal(out=var[:], in_=var[:])
                        inv = wk.tile([128, 1], F32, tag=f"inv{hf}")
                        nc.scalar.activation(out=inv[:], in_=var[:],
                                             func=AF.Sqrt)
                        sc = wk.tile([128, 1], F32, tag=f"sc{hf}")
                        nc.vector.tensor_tensor(out=sc[:], in0=g_t[:],
                                                in1=inv[:], op=ALU.mult)
                        sh = wk.tile([128, 1], F32, tag=f"sh{hf}")
                        nc.vector.tensor_tensor(out=sh[:], in0=mean[:],
                                                in1=sc[:], op=ALU.mult)
                        nc.vector.tensor_tensor(out=sh[:], in0=be_t[:],
                                                in1=sh[:], op=ALU.subtract)
                        sca.append(sc)
                        sha.append(sh)

                    # 4) BN1 apply + relu + matmul2 + BN2 stats (recompute mm1)
                    for ci, (c0, cw) in enumerate(chunks):
                        hrs = []
                        for hf in range(2):
                            pm = pmlp.tile([128, CH], F32, space="PSUM",
                                           tag=f"pm{hf}")
                            nc.tensor.matmul(
                                out=pm[:, :cw],
                                lhsT=d["w1"][:, hf * 128:(hf + 1) * 128],
                                rhs=xT[:, c0:c0 + cw], start=True, stop=True)
                            hr = wk.tile([128, CH], F32, tag=f"hr{hf}")
                            nc.scalar.activation(
                                out=hr[:, :cw], in_=pm[:, :cw], func=AF.Relu,
                                bias=sha[hf][:, :1], scale=sca[hf][:, :1])
                            hrs.append(hr)
                        pm2 = pmlp.tile([128, CH], F32, space="PSUM", tag="pm2",
                                        bufs=1)
                        nc.tensor.matmul(out=pm2[:, :cw], lhsT=d["w2a"][:],
                                         rhs=hrs[0][:, :cw], start=True,
                                         stop=False)
                        nc.tensor.matmul(out=pm2[:, :cw], lhsT=d["w2b"][:],
                                         rhs=hrs[1][:, :cw], start=False,
                                         stop=True)
                        nc.scalar.activation(
                            out=xT[:, c0:c0 + cw], in_=pm2[:, :cw],
                            func=AF.Copy,
                            accum_out=s2sum[:, ci:ci + 1])
                        sc2 = wk.tile([128, CH], F32, tag="sc2")
                        nc.scalar.activation(
                            out=sc2[:, :cw], in_=pm2[:, :cw], func=AF.Square,
                            accum_out=s2sq[:, ci:ci + 1])

                    st2 = wk.tile([128, 2], F32, tag="st2")
                    nc.vector.tensor_reduce(out=st2[:, 0:1], in_=s2sum[:, :],
                                            axis=mybir.AxisListType.X,
                                            op=ALU.add)
                    nc.vector.tensor_reduce(out=st2[:, 1:2], in_=s2sq[:, :],
                                            axis=mybir.AxisListType.X,
                                            op=ALU.add)
                    nc.sync.dma_start(out=s2i[:, :], in_=st2[:])
                    nc.gpsimd.collective_compute(
                        "AllReduce", ALU.add, replica_groups=enc_groups,
                        ins=[s2i.ap().opt()], outs=[s2o.ap().opt()])
                    sr2 = wk.tile([128, 2], F32, tag="sr2")
                    nc.sync.dma_start(out=sr2[:], in_=s2o[:, :])

                    mean = wk.tile([128, 1], F32, tag="mean2")
                    nc.vector.tensor_scalar_mul(out=mean[:], in0=sr2[:, 0:1],
                                                scalar1=inv_n)
                    var = wk.tile([128, 1], F32, tag="var2")
                    nc.vector.tensor_scalar_mul(out=var[:], in0=sr2[:, 1:2],
                                                scalar1=inv_n)
                    m2 = wk.tile([128, 1], F32, tag="m22")
                    nc.vector.tensor_tensor(out=m2[:], in0=mean[:], in1=mean[:],
                                            op=ALU.mult)
                    nc.vector.tensor_tensor(out=var[:], in0=var[:], in1=m2[:],
                                            op=ALU.subtract)
                    nc.vector.tensor_scalar_add(out=var[:], in0=var[:],
                                                scalar1=BN_EPS)
                    nc.vector.reciprocal(out=var[:], in_=var[:])
                    inv = wk.tile([128, 1], F32, tag="inv2")
                    nc.scalar.activation(out=inv[:], in_=var[:], func=AF.Sqrt)
                    sc2t = wk.tile([128, 1], F32, tag="sc2t")
                    nc.vector.tensor_tensor(out=sc2t[:], in0=d["g2"][:],
                                            in1=inv[:], op=ALU.mult)
                    sh2t = wk.tile([128, 1], F32, tag="sh2t")
                    nc.vector.tensor_tensor(out=sh2t[:], in0=mean[:],
                                            in1=sc2t[:], op=ALU.mult)
                    nc.vector.tensor_tensor(out=sh2t[:], in0=d["be2"][:],
                                            in1=sh2t[:], op=ALU.subtract)

                    for ci, (c0, cw) in enumerate(chunks):
                        nc.scalar.activation(out=xT[:, c0:c0 + cw],
                                             in_=xT[:, c0:c0 + cw],
                                             func=AF.Relu, bias=sh2t[:, :1],
                                             scale=sc2t[:, :1])

            # ---- pooling + head (fresh PSUM pools) ----
            with (
                tc.tile_pool(name="ppool", bufs=1, space="PSUM") as ppl,
                tc.tile_pool(name="ptr", bufs=2, space="PSUM") as ptr,
                tc.tile_pool(name="phd", bufs=2, space="PSUM") as phd,
            ):
                # tags: plg0/plg1 (1 bank each), ptp (2), ph (2) => 6 banks
                plg = [ppl.tile([128, 132], F32, space="PSUM", tag=f"plg{g}",
                                name=f"plg{g}")
                       for g in range(GH)]
                for w in range(NW):
                    w0 = w * WIN
                    ww = min(WIN, NSH - w0)
                    tp = ptr.tile([128, 128], F32, space="PSUM", tag="ptp")
                    nc.tensor.transpose(out=tp[:ww, :], in_=xT[:, w0:w0 + ww],
                                        identity=idn[:])
                    xw = wk.tile([128, 132], F32, tag="xw")
                    if ww < 128:
                        nc.vector.memset(xw[:], 0.0)
                    nc.scalar.copy(out=xw[:ww, :128], in_=tp[:ww, :])
                    nc.vector.tensor_copy(out=xw[:, 128:129], in_=onec[:, :1])
                    bt = wk.tile([128, B], F32, tag="bt")
                    b_ap = batf[:, w:w + 1]
                    nc.vector.tensor_tensor(
                        out=bt[:, :B],
                        in0=bass.AP(b_ap.tensor, b_ap.offset,
                                    [b_ap.ap[0], [0, B]]),
                        in1=iog[:, :B], op=ALU.is_equal)
                    for g in range(GH):
                        gw = min(128, B - g * 128)
                        nc.tensor.matmul(out=plg[g][:gw, 0:129],
                                         lhsT=bt[:, g * 128:g * 128 + gw],
                                         rhs=xw[:, :129], start=(w == 0),
                                         stop=(w == NW - 1))

                # combine shard-partial pooled sums/counts across the group
                pacc = wk.tile([128, GH * 132], F32, tag="pacc")
                nc.vector.memset(pacc[:], 0.0)
                for g in range(GH):
                    gw = min(128, B - g * 128)
                    nc.scalar.copy(out=pacc[:gw, g * 132:g * 132 + 129],
                                   in_=plg[g][:gw, :129])
                nc.sync.dma_start(out=pacc_in[:, :], in_=pacc[:])
                nc.gpsimd.collective_compute(
                    "AllReduce", ALU.add, replica_groups=enc_groups,
                    ins=[pacc_in.ap().opt()], outs=[pacc_out.ap().opt()])
                pr = wk.tile([128, GH * 132], F32, tag="pr")
                nc.sync.dma_start(out=pr[:], in_=pacc_out[:, :])

                embT = wk.tile([128, B], F32, tag="embT")
                for g in range(GH):
                    gw = min(128, B - g * 128)
                    invc = wk.tile([128, 1], F32, tag="invc")
                    nc.vector.tensor_scalar_max(
                        out=invc[:gw, :],
                        in0=pr[:gw, g * 132 + 128:g * 132 + 129],
                        scalar1=1.0)
                    nc.vector.reciprocal(out=invc[:gw, :], in_=invc[:gw, :])
                    pgs = wk.tile([128, 128], F32, tag="pgs")
                    nc.scalar.activation(out=pgs[:gw, :],
                                         in_=pr[:gw, g * 132:g * 132 + 128],
                                         func=AF.Copy, scale=invc[:gw, :1])
                    tpp = ptr.tile([128, 128], F32, space="PSUM", tag="ptp")
                    nc.tensor.transpose(out=tpp[:, :gw], in_=pgs[:gw, :],
                                        identity=idn[:gw, :gw])
                    nc.scalar.copy(out=embT[:, g * 128:g * 128 + gw],
                                   in_=tpp[:, :gw])

                nc.sync.dma_start(out=pool_in[:, :], in_=embT[:])
                nc.gpsimd.collective_compute(
                    "AllGather", ALU.bypass, replica_groups=pair_groups,
                    ins=[pool_in.ap().opt()], outs=[pool_out.ap().opt()])
                demb = wk.tile([128, B], F32, tag="demb")
                semb = wk.tile([128, B], F32, tag="semb")
                nc.sync.dma_start(out=demb[:], in_=pool_out[0:128, :])
                nc.sync.dma_start(out=semb[:], in_=pool_out[128:256, :])

                # temperature MLP
                ptm = phd.tile([32, B], F32, space="PSUM", tag="ph")
                nc.tensor.matmul(out=ptm[:], lhsT=ht["tw1"][:], rhs=trow[:],
                                 start=True, stop=True)
                t1 = wk.tile([32, B], F32, tag="t1")
                nc.scalar.activation(out=t1[:], in_=ptm[:], func=AF.Relu,
                                     bias=ht["tb1"][:, :1])
                ptm2 = phd.tile([32, B], F32, space="PSUM", tag="ph")
                nc.tensor.matmul(out=ptm2[:], lhsT=ht["tw2"][:], rhs=t1[:],
                                 start=True, stop=True)
                t2 = wk.tile([32, B], F32, tag="t2")
                nc.vector.tensor_scalar_add(out=t2[:], in0=ptm2[:],
                                            scalar1=ht["tb2"][:, :1])

                # prediction head
                ph1 = phd.tile([128, B], F32, space="PSUM", tag="ph")
                nc.tensor.matmul(out=ph1[:], lhsT=ht["pw1a"][:], rhs=demb[:],
                                 start=True, stop=False)
                nc.tensor.matmul(out=ph1[:], lhsT=ht["pw1b"][:], rhs=semb[:],
                                 start=False, stop=False)
                nc.tensor.matmul(out=ph1[:], lhsT=ht["pw1c"][:], rhs=t2[:],
                                 start=False, stop=True)
                h1s = wk.tile([128, B], F32, tag="h1s")
                nc.scalar.activation(out=h1s[:], in_=ph1[:], func=AF.Relu,
                                     bias=ht["pb1"][:, :1])
                ph2 = phd.tile([64, B], F32, space="PSUM", tag="ph")
                nc.tensor.matmul(out=ph2[:], lhsT=ht["pw2"][:], rhs=h1s[:],
                                 start=True, stop=True)
                h2s = wk.tile([64, B], F32, tag="h2s")
                nc.scalar.activation(out=h2s[:], in_=ph2[:], func=AF.Relu,
                                     bias=ht["pb2"][:, :1])
                ph3 = phd.tile([1, B], F32, space="PSUM", tag="ph")
                nc.tensor.matmul(out=ph3[:], lhsT=ht["pw3"][:], rhs=h2s[:],
                                 start=True, stop=True)
                oT = wk.tile([1, B], F32, tag="oT")
                nc.vector.tensor_scalar_add(out=oT[:], in0=ph3[:],
                                            scalar1=ht["pb3"][:, :1])
                nc.sync.dma_start(out=out_d[:, :], in_=oT[:])

    nc.compile()
    return nc


# ---------------------------------------------------------------------------
# Input packing
# ---------------------------------------------------------------------------

def _enc_param_maps(enc, NL):
    """Per-encoder named parameter arrays for the device program."""
    out = {
        "emb_w": np.asarray(enc["emb_w"], np.float32),
        "emb_b": np.asarray(enc["emb_b"], np.float32).reshape(HID, 1),
    }
    for l in range(NL):
        p = enc["layers"][l]
        w1 = np.asarray(p["w1"], np.float32)
        g1 = np.asarray(p["g1"], np.float32)
        be1 = np.asarray(p["be1"], np.float32)
        w2 = np.asarray(p["w2"], np.float32)
        eps = float(np.asarray(p["eps"]))
        out[f"w1_{l}"] = w1
        out[f"g1a_{l}"] = g1[:128].reshape(128, 1)
        out[f"g1b_{l}"] = g1[128:].reshape(128, 1)
        out[f"be1a_{l}"] = be1[:128].reshape(128, 1)
        out[f"be1b_{l}"] = be1[128:].reshape(128, 1)
        out[f"w2a_{l}"] = w2[:128]
        out[f"w2b_{l}"] = w2[128:]
        out[f"g2_{l}"] = np.asarray(p["g2"], np.float32).reshape(HID, 1)
        out[f"be2_{l}"] = np.asarray(p["be2"], np.float32).reshape(HID, 1)
        out[f"epsi_{l}"] = ((1.0 + eps) * np.eye(128)).astype(np.float32)
    return out


def make_in_maps(inputs, NL=4):
    drug_x = np.asarray(inputs["drug_x"], np.float32)
    solv_x = np.asarray(inputs["solvent_x"], np.float32)
    N, FIN = drug_x.shape
    assert N % GS == 0
    NSH = N // GS
    NW = -(-NSH // WIN)
    params = inputs["params"]
    temperature = np.asarray(inputs["temperature"], np.float32)
    B = temperature.shape[0]

    SPLIT = min(32768, N)
    if "split_override" in inputs:
        SPLIT = int(inputs["split_override"])
    enc_data = []
    counts_lo, counts_hi = [], []
    for key_x, key_e, key_b, key_p in (
        ("drug_x", "drug_edge_index", "drug_batch", "drug"),
        ("solvent_x", "solvent_edge_index", "solvent_batch", "solvent"),
    ):
        x = np.asarray(inputs[key_x], np.float32)
        shards = _shard_edges(np.asarray(inputs[key_e]), N, NSH)
        batch = np.asarray(inputs[key_b], np.int64)
        enc_data.append((x, shards, batch, params[key_p]))
        for ss, dd in shards:
            clo, chi = _window_stream_counts(ss, dd, NSH, SPLIT)
            counts_lo.append(clo)
            counts_hi.append(chi)
    T_lo = [int(max(-(-counts_lo[i][w] // 128) for i in range(len(counts_lo))))
            for w in range(NW)]
    T_hi = [int(max(-(-counts_hi[i][w] // 128) for i in range(len(counts_hi))))
            for w in range(NW)]

    io128 = np.tile(np.arange(WIN, dtype=np.float32)[None, :], (128, 1))
    iog = np.tile(np.arange(B, dtype=np.float32)[None, :], (128, 1))
    idn = np.eye(128, dtype=np.float32)
    ones = np.ones((128, 1), np.float32)
    trow = np.ascontiguousarray(temperature.reshape(1, B))

    hp = params["pred"]
    tp = params["temp"]
    pw1 = np.asarray(hp["w1"], np.float32)
    head = {
        "pw1a": pw1[:128], "pw1b": pw1[128:256], "pw1c": pw1[256:288],
        "pb1": np.asarray(hp["b1"], np.float32).reshape(128, 1),
        "pw2": np.asarray(hp["w2"], np.float32),
        "pb2": np.asarray(hp["b2"], np.float32).reshape(64, 1),
        "pw3": np.asarray(hp["w3"], np.float32),
        "pb3": np.asarray(hp["b3"], np.float32).reshape(1, 1),
        "tw1": np.asarray(tp["w1"], np.float32),
        "tb1": np.asarray(tp["b1"], np.float32).reshape(32, 1),
        "tw2": np.asarray(tp["w2"], np.float32),
        "tb2": np.asarray(tp["b2"], np.float32).reshape(32, 1),
    }

    in_maps = []
    for c in range(NCORES):
        e = c // GS
        s = c % GS
        x, shards, batch, enc_p = enc_data[e]
        src_a, dst_a = _build_edge_arrays(shards[s][0], shards[s][1], NSH,
                                          T_lo, T_hi, SPLIT)
        m = {
            "xin": np.ascontiguousarray(x[s * NSH:(s + 1) * NSH].T),
            "srcs": src_a,
            "dsts": dst_a,
            "batf": _batch_array(batch, NSH, s),
            "io128": io128, "iog": iog, "idn": idn, "ones": ones,
            "trow": trow,
        }
        m.update(_enc_param_maps(enc_p, NL))
        m.update(head)
        in_maps.append(m)

    cfg = {"NSH": NSH, "B": B, "FIN": FIN, "NL": NL,
           "T_lo": T_lo, "T_hi": T_hi, "SPLIT": SPLIT}
    return cfg, in_maps


_PROGRAM_CACHE = {}


def _get_program(cfg):
    key = (cfg["NSH"], cfg["B"], cfg["FIN"], cfg["NL"],
           tuple(cfg["T_lo"]), tuple(cfg["T_hi"]), cfg["SPLIT"])
    if key not in _PROGRAM_CACHE:
        _PROGRAM_CACHE[key] = build_program(cfg)
    return _PROGRAM_CACHE[key]


def kernel(**inputs) -> np.ndarray:
    from concourse.bass_utils import run_bass_kernel_spmd
    cfg, in_maps = make_in_maps(inputs)
    nc = _get_program(cfg)
    res = run_bass_kernel_spmd(nc, in_maps, core_ids=list(range(NCORES)))
    out = np.asarray(res.results[0]["out"], np.float32)
    return out.reshape(-1, 1)


# revision 16
# speedup vs baseline: 1.0257x; 1.0257x over previous
"""Trainium2 Bass kernel for the BigSolDB pretrain model (two GIN encoders +
fusion head), distributed over 8 NeuronCores.

Sharding: cores 0-3 run the drug encoder, cores 4-7 the solvent encoder
(identical SPMD program, different per-core data).  Within each group of 4,
nodes are sharded into 4 contiguous ranges; each core handles the edges whose
destination falls in its range.  Per GIN layer the full node-feature table is
rebuilt with an AllGather so every core can gather arbitrary source rows, and
batch-norm statistics are combined with a small AllReduce.  Pooled graph
embeddings are exchanged across the two groups and the fusion head is computed
redundantly on every core.

The edge aggregation (segment_sum of x[src] by dst) runs as:
  - batched indirect DMA row gathers from the HBM feature table, and
  - PE matmuls against on-chip-built one-hot (edge -> dst slot) matrices,
    accumulating each 128-wide dst window in PSUM.
"""

import numpy as np

import concourse.bass as bass
import concourse.bacc as bacc
import concourse.mybir as mybir
import concourse.tile as tile

F32 = mybir.dt.float32
I32 = mybir.dt.int32
AF = mybir.ActivationFunctionType
ALU = mybir.AluOpType

HID = 128
HID2 = 256
NCORES = 8
GS = 4          # cores per encoder group
WIN = 128       # dst-window width (one-hot matmul N dim)
CH = 512        # node-chunk width for the MLP phases
BN_EPS = 1e-5


# ---------------------------------------------------------------------------
# Host-side sharding / scheduling
# ---------------------------------------------------------------------------

def _shard_edges(edge_index, n_nodes, nsh):
    """Split edges by dst shard; per shard return (src, dst_local) sorted by
    dst_local."""
    src = np.asarray(edge_index[0], dtype=np.int64)
    dst = np.asarray(edge_index[1], dtype=np.int64)
    shards = []
    for s in range(GS):
        lo, hi = s * nsh, (s + 1) * nsh
        m = (dst >= lo) & (dst < hi)
        ss, dd = src[m], dst[m] - lo
        order = np.argsort(dd, kind="stable")
        shards.append((ss[order], dd[order]))
    return shards


def _window_counts(dst_local, nsh):
    nw = -(-nsh // WIN)
    return np.bincount(dst_local // WIN, minlength=nw)


def _window_stream_counts(src, dst_local, nsh, split):
    """Per window: (lo_count, hi_count) by src < split."""
    nw = -(-nsh // WIN)
    w_of_edge = dst_local // WIN
    lo = src < split
    clo = np.bincount(w_of_edge[lo], minlength=nw)
    chi = np.bincount(w_of_edge[~lo], minlength=nw)
    return clo, chi


GW = 2  # windows per gather-group


def _grid_layout(T_lo, T_hi):
    """Grouped grid: per group of GW windows, lo blocks then hi blocks.
    Returns (groups, NT): groups = list of (windows, lo_bases, hi_bases,
    gbase, gtiles)."""
    nw = len(T_lo)
    groups = []
    base = 0
    for g0 in range(0, nw, GW):
        ws = list(range(g0, min(g0 + GW, nw)))
        gbase = base
        lo_bases, hi_bases = {}, {}
        for w in ws:
            lo_bases[w] = base
            base += T_lo[w]
        for w in ws:
            hi_bases[w] = base
            base += T_hi[w]
        groups.append((ws, lo_bases, hi_bases, gbase, base - gbase))
    return groups, base


def _build_edge_arrays(src, dst_local, nsh, T_lo, T_hi, split):
    """Pack edges into the shared grouped (lo/hi) tile grid.

    Returns idx16 [128, NT*8] int16 (dma_gather wrapped layout, replicated
    into the 8 Q7-core stripes) and dst_f [128, NT] float32 (-1 in pad
    slots)."""
    nw = len(T_lo)
    groups, NT = _grid_layout(T_lo, T_hi)
    idx16 = np.zeros((128, NT * 8), dtype=np.int16)
    dst_a = np.full((128, NT), -1.0, dtype=np.float32)
    w_of_edge = dst_local // WIN
    starts = np.searchsorted(w_of_edge, np.arange(nw))
    ends = np.searchsorted(w_of_edge, np.arange(nw) + 1)

    def put(base, Ts, es, ed, w):
        cnt = len(es)
        flat_s = np.zeros(Ts * 128, dtype=np.int16)
        flat_d = np.full(Ts * 128, -1.0, dtype=np.float32)
        flat_s[:cnt] = es.astype(np.int16)
        flat_d[:cnt] = ed - w * WIN
        dst_a[:, base:base + Ts] = flat_d.reshape(Ts, 128).T
        k = np.arange(Ts * 128)
        cols = base * 8 + k // 16
        for c in range(8):
            idx16[16 * c + (k % 16), cols] = flat_s

    for ws, lo_bases, hi_bases, gbase, gtiles in groups:
        for w in ws:
            sl = slice(starts[w], ends[w])
            es, ed = src[sl], dst_local[sl]
            lo_m = es < split
            if T_lo[w]:
                put(lo_bases[w], T_lo[w], es[lo_m], ed[lo_m], w)
            if T_hi[w]:
                put(hi_bases[w], T_hi[w], es[~lo_m] - split, ed[~lo_m], w)
    return idx16, dst_a


def _batch_array(batch, nsh, s):
    """[128, NW] float32 graph ids of this shard's nodes, -1 in pad slots."""
    nw = -(-nsh // WIN)
    out = np.full((128, nw), -1.0, dtype=np.float32)
    vals = np.asarray(batch[s * nsh:(s + 1) * nsh], dtype=np.float32)
    pad = nw * WIN - len(vals)
    if pad:
        vals = np.concatenate([vals, np.full(pad, -1.0, np.float32)])
    out[:, :] = vals.reshape(nw, 128).T
    return out


def _chunks(n, ch):
    return [(c0, min(ch, n - c0)) for c0 in range(0, n, ch)]


# ---------------------------------------------------------------------------
# Program builder (one SPMD program shared by all 8 cores)
# ---------------------------------------------------------------------------

def build_program(cfg):
    NSH = cfg["NSH"]
    N = GS * NSH
    NW = -(-NSH // WIN)
    SPLIT = cfg["SPLIT"]
    T_lo = cfg["T_lo"]
    T_hi = cfg["T_hi"]
    groups, NT = _grid_layout(T_lo, T_hi)
    Tmax = max(1, max(T_lo[w] + T_hi[w] for w in range(NW)))
    GTmax = max(1, max(g[4] for g in groups))
    HAS_HI = N > SPLIT
    B = cfg["B"]
    FIN = cfg["FIN"]
    NL = cfg["NL"]
    GH = -(-B // 128)            # graph-halves (1 if B<=128, else 2)
    chunks = _chunks(NSH, CH)
    NCH = len(chunks)
    inv_n = 1.0 / float(N)
    enc_groups = [list(range(GS)), list(range(GS, 2 * GS))]
    pair_groups = [[c, c + GS] for c in range(GS)]

    nc = bacc.Bacc("TRN2", target_bir_lowering=False, debug=False,
                   num_devices=NCORES)

    def din(name, shape, dtype=F32):
        return nc.dram_tensor(name, shape, dtype, kind="ExternalInput")

    xin = din("xin", [FIN, NSH])
    srcs_d = din("srcs", [128, NT * 8], mybir.dt.int16)
    dsts_d = din("dsts", [128, NT])
    batf_d = din("batf", [128, NW])
    io128_d = din("io128", [128, WIN])
    iog_d = din("iog", [128, B])
    idn_d = din("idn", [128, 128])
    ones_d = din("ones", [128, 1])
    trow_d = din("trow", [1, B])

    embw_d = din("emb_w", [FIN, HID])
    embb_d = din("emb_b", [HID, 1])
    lp = []
    for l in range(NL):
        lp.append({k: din(f"{k}_{l}", shp) for k, shp in [
            ("w1", [HID, HID2]),
            ("g1a", [128, 1]), ("g1b", [128, 1]),
            ("be1a", [128, 1]), ("be1b", [128, 1]),
            ("w2a", [128, HID]), ("w2b", [128, HID]),
            ("g2", [HID, 1]), ("be2", [HID, 1]),
            ("epsi", [128, 128]),
        ]})
    hp = {k: din(k, shp) for k, shp in [
        ("pw1a", [128, 128]), ("pw1b", [128, 128]), ("pw1c", [32, 128]),
        ("pb1", [128, 1]), ("pw2", [128, 64]), ("pb2", [64, 1]),
        ("pw3", [64, 1]), ("pb3", [1, 1]),
        ("tw1", [1, 32]), ("tb1", [32, 1]), ("tw2", [32, 32]), ("tb2", [32, 1]),
    ]}

    out_d = nc.dram_tensor("out", [1, B], F32, kind="ExternalOutput")

    # internal DRAM
    xtab_in = nc.dram_tensor("xtab_in", [NSH, HID], F32)
    x_table = nc.dram_tensor("x_table", [N, HID], F32)
    xtab_hi = (nc.dram_tensor("xtab_hi", [N - SPLIT, HID], F32)
               if HAS_HI else None)
    st_bufs = []
    for l in range(NL):
        st_bufs.append((
            nc.dram_tensor(f"s1i_{l}", [128, 4], F32),
            nc.dram_tensor(f"s1o_{l}", [128, 4], F32),
            nc.dram_tensor(f"s2i_{l}", [128, 2], F32),
            nc.dram_tensor(f"s2o_{l}", [128, 2], F32),
        ))
    GH_ = -(-B // 128)
    pacc_in = nc.dram_tensor("pacc_in", [128, GH_ * 132], F32)
    pacc_out = nc.dram_tensor("pacc_out", [128, GH_ * 132], F32)
    pool_in = nc.dram_tensor("pool_in", [128, B], F32)
    pool_out = nc.dram_tensor("pool_out", [256, B], F32)

    with tile.TileContext(nc) as tc:
        with (
            tc.tile_pool(name="persist", bufs=1) as pp,
            tc.tile_pool(name="wrk", bufs=3) as wk,
            tc.tile_pool(name="gat", bufs=3) as gp,
        ):
            # ---- persistent loads ----
            xT = pp.tile([128, NSH], F32, tag="xT")
            dsts = pp.tile([128, NT], F32, tag="dsts")
            batf = pp.tile([128, NW], F32, tag="batf")
            io128 = pp.tile([128, WIN], F32, tag="io128")
            iog = pp.tile([128, B], F32, tag="iog")
            idn = pp.tile([128, 128], F32, tag="idn")
            onec = pp.tile([128, 1], F32, tag="onec")
            trow = pp.tile([1, B], F32, tag="trow")
            ssum = pp.tile([128, 2 * NCH], F32, tag="ssum")
            ssq = pp.tile([128, 2 * NCH], F32, tag="ssq")
            s2sum = pp.tile([128, NCH], F32, tag="s2sum")
            s2sq = pp.tile([128, NCH], F32, tag="s2sq")

            nc.sync.dma_start(out=dsts[:], in_=dsts_d[:, :])
            nc.sync.dma_start(out=batf[:], in_=batf_d[:, :])
            nc.sync.dma_start(out=io128[:], in_=io128_d[:, :])
            nc.sync.dma_start(out=iog[:], in_=iog_d[:, :])
            nc.sync.dma_start(out=idn[:], in_=idn_d[:, :])
            nc.sync.dma_start(out=onec[:], in_=ones_d[:, :])
            nc.sync.dma_start(out=trow[:], in_=trow_d[:, :])

            embw = pp.tile([FIN, HID], F32, tag="embw")
            embb = pp.tile([HID, 1], F32, tag="embb")
            nc.sync.dma_start(out=embw[:], in_=embw_d[:, :])
            nc.sync.dma_start(out=embb[:], in_=embb_d[:, :])
            lt = []
            for l in range(NL):
                d = {}
                for k, h in lp[l].items():
                    d[k] = pp.tile(list(h.shape), F32, tag=f"{k}_{l}",
                                   name=f"{k}_{l}_t")
                    nc.sync.dma_start(out=d[k][:], in_=h[:, :])
                lt.append(d)
            ht = {}
            for k, h in hp.items():
                ht[k] = pp.tile(list(h.shape), F32, tag=k, name=f"{k}_t")
                nc.sync.dma_start(out=ht[k][:], in_=h[:, :])

            with (
                tc.tile_pool(name="pq", bufs=3, space="PSUM") as pqp,
                tc.tile_pool(name="pmlp", bufs=2, space="PSUM") as pmlp,
            ):
                # ---- embedding: xT = emb_w.T @ xin (+ emb_b) ----
                for ci, (c0, cw) in enumerate(chunks):
                    xc = wk.tile([FIN, CH], F32, tag="xc")
                    nc.sync.dma_start(out=xc[:, :cw], in_=xin[:, c0:c0 + cw])
                    pe = pmlp.tile([128, CH], F32, space="PSUM", tag="pm0")
                    nc.tensor.matmul(out=pe[:, :cw], lhsT=embw[:],
                                     rhs=xc[:, :cw], start=True, stop=True)
                    nc.vector.tensor_scalar_add(out=xT[:, c0:c0 + cw],
                                                in0=pe[:, :cw],
                                                scalar1=embb[:, :1])

                # ---- GIN layers ----
                for l in range(NL):
                    d = lt[l]
                    s1i, s1o, s2i, s2o = st_bufs[l]

                    # 1) rebuild the gather table: x_table <- AllGather(x_loc)
                    for w in range(NW):
                        w0 = w * WIN
                        ww = min(WIN, NSH - w0)
                        tp = pqp.tile([128, 128], F32, space="PSUM", tag="pq")
                        nc.tensor.transpose(out=tp[:ww, :],
                                            in_=xT[:, w0:w0 + ww],
                                            identity=idn[:])
                        ts = wk.tile([128, 128], F32, tag="tts")
                        nc.scalar.copy(out=ts[:ww, :], in_=tp[:ww, :])
                        nc.sync.dma_start(out=xtab_in[w0:w0 + ww, :],
                                          in_=ts[:ww, :])
                    nc.gpsimd.collective_compute(
                        "AllGather", ALU.bypass, replica_groups=enc_groups,
                        ins=[xtab_in.ap().opt()], outs=[x_table.ap().opt()])
                    if HAS_HI:
                        nc.sync.dma_start(out=xtab_hi[:, :],
                                          in_=x_table[SPLIT:, :])

                    # 2) aggregate + h = (1+eps)*x + agg   (in place on xT)
                    for ws, lo_bases, hi_bases, gbase, gtiles in groups:
                        GTl = sum(T_lo[w] for w in ws)
                        GTh = sum(T_hi[w] for w in ws)
                        gt = None
                        if gtiles > 0:
                            idxw = gp.tile([128, GTmax * 8], mybir.dt.int16,
                                           tag="idxw", bufs=2)
                            nc.sync.dma_start(
                                out=idxw[:, :gtiles * 8],
                                in_=srcs_d[:, gbase * 8:(gbase + gtiles) * 8])
                            gt = gp.tile([128, GTmax * HID], F32, tag="gt",
                                         bufs=2)
                            if GTl > 0:
                                nc.gpsimd.dma_gather(
                                    out_ap=gt[:, :GTl * HID].rearrange(
                                        "p (t d) -> p t d", d=HID),
                                    in_ap=x_table[:, :],
                                    idxs_ap=idxw[:, :GTl * 8],
                                    num_idxs=GTl * 128,
                                    num_idxs_reg=GTl * 128,
                                    elem_size=HID,
                                    single_packet=False)
                            if GTh > 0:
                                nc.gpsimd.dma_gather(
                                    out_ap=gt[:, GTl * HID:gtiles * HID
                                              ].rearrange(
                                        "p (t d) -> p t d", d=HID),
                                    in_ap=xtab_hi[:, :],
                                    idxs_ap=idxw[:, GTl * 8:gtiles * 8],
                                    num_idxs=GTh * 128,
                                    num_idxs_reg=GTh * 128,
                                    elem_size=HID,
                                    single_packet=False)
                        for w in ws:
                            Tl, Th = T_lo[w], T_hi[w]
                            Tw = Tl + Th
                            w0 = w * WIN
                            ww = min(WIN, NSH - w0)
                            pag = pqp.tile([128, WIN], F32, space="PSUM",
                                           tag="pq")
                            nc.tensor.matmul(out=pag[:, :ww],
                                             lhsT=d["epsi"][:],
                                             rhs=xT[:, w0:w0 + ww],
                                             start=True, stop=(Tw == 0))
                            if Tw > 0:
                                mt = gp.tile([128, Tmax * WIN], F32, tag="mt",
                                             bufs=2)
                                for si, (sb, Ts) in enumerate(
                                        ((lo_bases[w], Tl), (hi_bases[w], Th))):
                                    if Ts == 0:
                                        continue
                                    moff = 0 if si == 0 else Tl * ww
                                    din_ap = dsts[:, sb:sb + Ts]
                                    in0 = bass.AP(
                                        din_ap.tensor, din_ap.offset,
                                        [din_ap.ap[0], [din_ap.ap[1][0], Ts],
                                         [0, ww]])
                                    io_ap = io128[:, :ww]
                                    in1 = bass.AP(
                                        io_ap.tensor, io_ap.offset,
                                        [io_ap.ap[0], [0, Ts], [1, ww]])
                                    nc.vector.tensor_tensor(
                                        out=mt[:, moff:moff + Ts * ww
                                               ].rearrange(
                                            "p (t w) -> p t w", t=Ts),
                                        in0=in0, in1=in1, op=ALU.is_equal)
                                for t in range(Tw):
                                    grid_col = (lo_bases[w] + t if t < Tl
                                                else hi_bases[w] + (t - Tl))
                                    nc.tensor.matmul(
                                        out=pag[:, :ww],
                                        lhsT=gt[:, (grid_col - gbase) * HID:
                                                (grid_col - gbase + 1) * HID],
                                        rhs=mt[:, t * ww:(t + 1) * ww],
                                        start=False, stop=(t == Tw - 1))
                            nc.vector.tensor_copy(out=xT[:, w0:w0 + ww],
                                                  in_=pag[:, :ww])

                    # 3) first-pass matmul1 for BN1 statistics
                    for ci, (c0, cw) in enumerate(chunks):
                        for hf in range(2):
                            pm = pmlp.tile([128, CH], F32, space="PSUM",
                                           tag=f"pm{hf}")
                            nc.tensor.matmul(
                                out=pm[:, :cw],
                                lhsT=d["w1"][:, hf * 128:(hf + 1) * 128],
                                rhs=xT[:, c0:c0 + cw], start=True, stop=True)
                            sc1 = wk.tile([128, CH], F32, tag="sc1")
                            nc.scalar.activation(
                                out=sc1[:, :cw], in_=pm[:, :cw], func=AF.Copy,
                                accum_out=ssum[:, hf * NCH + ci:hf * NCH + ci + 1])
                            sc2 = wk.tile([128, CH], F32, tag="sc2")
                            nc.scalar.activation(
                                out=sc2[:, :cw], in_=pm[:, :cw], func=AF.Square,
                                accum_out=ssq[:, hf * NCH + ci:hf * NCH + ci + 1])

                    st = wk.tile([128, 4], F32, tag="st")
                    for hf in range(2):
                        nc.vector.tensor_reduce(
                            out=st[:, hf:hf + 1],
                            in_=ssum[:, hf * NCH:(hf + 1) * NCH],
                            axis=mybir.AxisListType.X, op=ALU.add)
                        nc.vector.tensor_reduce(
                            out=st[:, 2 + hf:3 + hf],
                            in_=ssq[:, hf * NCH:(hf + 1) * NCH],
                            axis=mybir.AxisListType.X, op=ALU.add)
                    nc.sync.dma_start(out=s1i[:, :], in_=st[:])
                    nc.gpsimd.collective_compute(
                        "AllReduce", ALU.add, replica_groups=enc_groups,
                        ins=[s1i.ap().opt()], outs=[s1o.ap().opt()])
                    sr = wk.tile([128, 4], F32, tag="sr")
                    nc.sync.dma_start(out=sr[:], in_=s1o[:, :])

                    sca, sha = [], []
                    for hf in range(2):
                        g_t = d["g1a"] if hf == 0 else d["g1b"]
                        be_t = d["be1a"] if hf == 0 else d["be1b"]
                        mean = wk.tile([128, 1], F32, tag=f"mean{hf}")
                        nc.vector.tensor_scalar_mul(
                            out=mean[:], in0=sr[:, hf:hf + 1], scalar1=inv_n)
                        var = wk.tile([128, 1], F32, tag=f"var{hf}")
                        # var = sq/n - mean^2 ; then + eps
                        nc.vector.tensor_scalar_mul(
                            out=var[:], in0=sr[:, 2 + hf:3 + hf], scalar1=inv_n)
                        m2 = wk.tile([128, 1], F32, tag=f"m2{hf}")
                        nc.vector.tensor_tensor(out=m2[:], in0=mean[:],
                                                in1=mean[:], op=ALU.mult)
                        nc.vector.tensor_tensor(out=var[:], in0=var[:],
                                                in1=m2[:], op=ALU.subtract)
                        nc.vector.tensor_scalar_add(out=var[:], in0=var[:],
                                                    scalar1=BN_EPS)
                        nc.vector.reciprocal(out=var[:], in_=var[:])
                        inv = wk.tile([128, 1], F32, tag=f"inv{hf}")
                        nc.scalar.activation(out=inv[:], in_=var[:],
                                             func=AF.Sqrt)
                        sc = wk.tile([128, 1], F32, tag=f"sc{hf}")
                        nc.vector.tensor_tensor(out=sc[:], in0=g_t[:],
                                                in1=inv[:], op=ALU.mult)
                        sh = wk.tile([128, 1], F32, tag=f"sh{hf}")
                        nc.vector.tensor_tensor(out=sh[:], in0=mean[:],
                                                in1=sc[:], op=ALU.mult)
                        nc.vector.tensor_tensor(out=sh[:], in0=be_t[:],
                                                in1=sh[:], op=ALU.subtract)
                        sca.append(sc)
                        sha.append(sh)

                    # 4) BN1 apply + relu + matmul2 + BN2 stats (recompute mm1)
                    for ci, (c0, cw) in enumerate(chunks):
                        hrs = []
                        for hf in range(2):
                            pm = pmlp.tile([128, CH], F32, space="PSUM",
                                           tag=f"pm{hf}")
                            nc.tensor.matmul(
                                out=pm[:, :cw],
                                lhsT=d["w1"][:, hf * 128:(hf + 1) * 128],
                                rhs=xT[:, c0:c0 + cw], start=True, stop=True)
                            hr = wk.tile([128, CH], F32, tag=f"hr{hf}")
                            nc.scalar.activation(
                                out=hr[:, :cw], in_=pm[:, :cw], func=AF.Relu,
                                bias=sha[hf][:, :1], scale=sca[hf][:, :1])
                            hrs.append(hr)
                        pm2 = pmlp.tile([128, CH], F32, space="PSUM", tag="pm2",
                                        bufs=1)
                        nc.tensor.matmul(out=pm2[:, :cw], lhsT=d["w2a"][:],
                                         rhs=hrs[0][:, :cw], start=True,
                                         stop=False)
                        nc.tensor.matmul(out=pm2[:, :cw], lhsT=d["w2b"][:],
                                         rhs=hrs[1][:, :cw], start=False,
                                         stop=True)
                        nc.scalar.activation(
                            out=xT[:, c0:c0 + cw], in_=pm2[:, :cw],
                            func=AF.Copy,
                            accum_out=s2sum[:, ci:ci + 1])
                        sc2 = wk.tile([128, CH], F32, tag="sc2")
                        nc.scalar.activation(
                            out=sc2[:, :cw], in_=pm2[:, :cw], func=AF.Square,
                            accum_out=s2sq[:, ci:ci + 1])

                    st2 = wk.tile([128, 2], F32, tag="st2")
                    nc.vector.tensor_reduce(out=st2[:, 0:1], in_=s2sum[:, :],
                                            axis=mybir.AxisListType.X,
                                            op=ALU.add)
                    nc.vector.tensor_reduce(out=st2[:, 1:2], in_=s2sq[:, :],
                                            axis=mybir.AxisListType.X,
                                            op=ALU.add)
                    nc.sync.dma_start(out=s2i[:, :], in_=st2[:])
                    nc.gpsimd.collective_compute(
                        "AllReduce", ALU.add, replica_groups=enc_groups,
                        ins=[s2i.ap().opt()], outs=[s2o.ap().opt()])
                    sr2 = wk.tile([128, 2], F32, tag="sr2")
                    nc.sync.dma_start(out=sr2[:], in_=s2o[:, :])

                    mean = wk.tile([128, 1], F32, tag="mean2")
                    nc.vector.tensor_scalar_mul(out=mean[:], in0=sr2[:, 0:1],
                                                scalar1=inv_n)
                    var = wk.tile([128, 1], F32, tag="var2")
                    nc.vector.tensor_scalar_mul(out=var[:], in0=sr2[:, 1:2],
                                                scalar1=inv_n)
                    m2 = wk.tile([128, 1], F32, tag="m22")
                    nc.vector.tensor_tensor(out=m2[:], in0=mean[:], in1=mean[:],
                                            op=ALU.mult)
                    nc.vector.tensor_tensor(out=var[:], in0=var[:], in1=m2[:],
                                            op=ALU.subtract)
                    nc.vector.tensor_scalar_add(out=var[:], in0=var[:],
                                                scalar1=BN_EPS)
                    nc.vector.reciprocal(out=var[:], in_=var[:])
                    inv = wk.tile([128, 1], F32, tag="inv2")
                    nc.scalar.activation(out=inv[:], in_=var[:], func=AF.Sqrt)
                    sc2t = wk.tile([128, 1], F32, tag="sc2t")
                    nc.vector.tensor_tensor(out=sc2t[:], in0=d["g2"][:],
                                            in1=inv[:], op=ALU.mult)
                    sh2t = wk.tile([128, 1], F32, tag="sh2t")
                    nc.vector.tensor_tensor(out=sh2t[:], in0=mean[:],
                                            in1=sc2t[:], op=ALU.mult)
                    nc.vector.tensor_tensor(out=sh2t[:], in0=d["be2"][:],
                                            in1=sh2t[:], op=ALU.subtract)

                    for ci, (c0, cw) in enumerate(chunks):
                        nc.scalar.activation(out=xT[:, c0:c0 + cw],
                                             in_=xT[:, c0:c0 + cw],
                                             func=AF.Relu, bias=sh2t[:, :1],
                                             scale=sc2t[:, :1])

            # ---- pooling + head (fresh PSUM pools) ----
            with (
                tc.tile_pool(name="ppool", bufs=1, space="PSUM") as ppl,
                tc.tile_pool(name="ptr", bufs=2, space="PSUM") as ptr,
                tc.tile_pool(name="phd", bufs=2, space="PSUM") as phd,
            ):
                # tags: plg0/plg1 (1 bank each), ptp (2), ph (2) => 6 banks
                plg = [ppl.tile([128, 132], F32, space="PSUM", tag=f"plg{g}",
                                name=f"plg{g}")
                       for g in range(GH)]
                for w in range(NW):
                    w0 = w * WIN
                    ww = min(WIN, NSH - w0)
                    tp = ptr.tile([128, 128], F32, space="PSUM", tag="ptp")
                    nc.tensor.transpose(out=tp[:ww, :], in_=xT[:, w0:w0 + ww],
                                        identity=idn[:])
                    xw = wk.tile([128, 132], F32, tag="xw")
                    if ww < 128:
                        nc.vector.memset(xw[:], 0.0)
                    nc.scalar.copy(out=xw[:ww, :128], in_=tp[:ww, :])
                    nc.vector.tensor_copy(out=xw[:, 128:129], in_=onec[:, :1])
                    bt = wk.tile([128, B], F32, tag="bt")
                    b_ap = batf[:, w:w + 1]
                    nc.vector.tensor_tensor(
                        out=bt[:, :B],
                        in0=bass.AP(b_ap.tensor, b_ap.offset,
                                    [b_ap.ap[0], [0, B]]),
                        in1=iog[:, :B], op=ALU.is_equal)
                    for g in range(GH):
                        gw = min(128, B - g * 128)
                        nc.tensor.matmul(out=plg[g][:gw, 0:129],
                                         lhsT=bt[:, g * 128:g * 128 + gw],
                                         rhs=xw[:, :129], start=(w == 0),
                                         stop=(w == NW - 1))

                # combine shard-partial pooled sums/counts across the group
                pacc = wk.tile([128, GH * 132], F32, tag="pacc")
                nc.vector.memset(pacc[:], 0.0)
                for g in range(GH):
                    gw = min(128, B - g * 128)
                    nc.scalar.copy(out=pacc[:gw, g * 132:g * 132 + 129],
                                   in_=plg[g][:gw, :129])
                nc.sync.dma_start(out=pacc_in[:, :], in_=pacc[:])
                nc.gpsimd.collective_compute(
                    "AllReduce", ALU.add, replica_groups=enc_groups,
                    ins=[pacc_in.ap().opt()], outs=[pacc_out.ap().opt()])
                pr = wk.tile([128, GH * 132], F32, tag="pr")
                nc.sync.dma_start(out=pr[:], in_=pacc_out[:, :])

                embT = wk.tile([128, B], F32, tag="embT")
                for g in range(GH):
                    gw = min(128, B - g * 128)
                    invc = wk.tile([128, 1], F32, tag="invc")
                    nc.vector.tensor_scalar_max(
                        out=invc[:gw, :],
                        in0=pr[:gw, g * 132 + 128:g * 132 + 129],
                        scalar1=1.0)
                    nc.vector.reciprocal(out=invc[:gw, :], in_=invc[:gw, :])
                    pgs = wk.tile([128, 128], F32, tag="pgs")
                    nc.scalar.activation(out=pgs[:gw, :],
                                         in_=pr[:gw, g * 132:g * 132 + 128],
                                         func=AF.Copy, scale=invc[:gw, :1])
                    tpp = ptr.tile([128, 128], F32, space="PSUM", tag="ptp")
                    nc.tensor.transpose(out=tpp[:, :gw], in_=pgs[:gw, :],
                                        identity=idn[:gw, :gw])
                    nc.scalar.copy(out=embT[:, g * 128:g * 128 + gw],
                                   in_=tpp[:, :gw])

                nc.sync.dma_start(out=pool_in[:, :], in_=embT[:])
                nc.gpsimd.collective_compute(
                    "AllGather", ALU.bypass, replica_groups=pair_groups,
                    ins=[pool_in.ap().opt()], outs=[pool_out.ap().opt()])
                demb = wk.tile([128, B], F32, tag="demb")
                semb = wk.tile([128, B], F32, tag="semb")
                nc.sync.dma_start(out=demb[:], in_=pool_out[0:128, :])
                nc.sync.dma_start(out=semb[:], in_=pool_out[128:256, :])

                # temperature MLP
                ptm = phd.tile([32, B], F32, space="PSUM", tag="ph")
                nc.tensor.matmul(out=ptm[:], lhsT=ht["tw1"][:], rhs=trow[:],
                                 start=True, stop=True)
                t1 = wk.tile([32, B], F32, tag="t1")
                nc.scalar.activation(out=t1[:], in_=ptm[:], func=AF.Relu,
                                     bias=ht["tb1"][:, :1])
                ptm2 = phd.tile([32, B], F32, space="PSUM", tag="ph")
                nc.tensor.matmul(out=ptm2[:], lhsT=ht["tw2"][:], rhs=t1[:],
                                 start=True, stop=True)
                t2 = wk.tile([32, B], F32, tag="t2")
                nc.vector.tensor_scalar_add(out=t2[:], in0=ptm2[:],
                                            scalar1=ht["tb2"][:, :1])

                # prediction head
                ph1 = phd.tile([128, B], F32, space="PSUM", tag="ph")
                nc.tensor.matmul(out=ph1[:], lhsT=ht["pw1a"][:], rhs=demb[:],
                                 start=True, stop=False)
                nc.tensor.matmul(out=ph1[:], lhsT=ht["pw1b"][:], rhs=semb[:],
                                 start=False, stop=False)
                nc.tensor.matmul(out=ph1[:], lhsT=ht["pw1c"][:], rhs=t2[:],
                                 start=False, stop=True)
                h1s = wk.tile([128, B], F32, tag="h1s")
                nc.scalar.activation(out=h1s[:], in_=ph1[:], func=AF.Relu,
                                     bias=ht["pb1"][:, :1])
                ph2 = phd.tile([64, B], F32, space="PSUM", tag="ph")
                nc.tensor.matmul(out=ph2[:], lhsT=ht["pw2"][:], rhs=h1s[:],
                                 start=True, stop=True)
                h2s = wk.tile([64, B], F32, tag="h2s")
                nc.scalar.activation(out=h2s[:], in_=ph2[:], func=AF.Relu,
                                     bias=ht["pb2"][:, :1])
                ph3 = phd.tile([1, B], F32, space="PSUM", tag="ph")
                nc.tensor.matmul(out=ph3[:], lhsT=ht["pw3"][:], rhs=h2s[:],
                                 start=True, stop=True)
                oT = wk.tile([1, B], F32, tag="oT")
                nc.vector.tensor_scalar_add(out=oT[:], in0=ph3[:],
                                            scalar1=ht["pb3"][:, :1])
                nc.sync.dma_start(out=out_d[:, :], in_=oT[:])

    nc.compile()
    return nc


# ---------------------------------------------------------------------------
# Input packing
# ---------------------------------------------------------------------------

def _enc_param_maps(enc, NL):
    """Per-encoder named parameter arrays for the device program."""
    out = {
        "emb_w": np.asarray(enc["emb_w"], np.float32),
        "emb_b": np.asarray(enc["emb_b"], np.float32).reshape(HID, 1),
    }
    for l in range(NL):
        p = enc["layers"][l]
        w1 = np.asarray(p["w1"], np.float32)
        g1 = np.asarray(p["g1"], np.float32)
        be1 = np.asarray(p["be1"], np.float32)
        w2 = np.asarray(p["w2"], np.float32)
        eps = float(np.asarray(p["eps"]))
        out[f"w1_{l}"] = w1
        out[f"g1a_{l}"] = g1[:128].reshape(128, 1)
        out[f"g1b_{l}"] = g1[128:].reshape(128, 1)
        out[f"be1a_{l}"] = be1[:128].reshape(128, 1)
        out[f"be1b_{l}"] = be1[128:].reshape(128, 1)
        out[f"w2a_{l}"] = w2[:128]
        out[f"w2b_{l}"] = w2[128:]
        out[f"g2_{l}"] = np.asarray(p["g2"], np.float32).reshape(HID, 1)
        out[f"be2_{l}"] = np.asarray(p["be2"], np.float32).reshape(HID, 1)
        out[f"epsi_{l}"] = ((1.0 + eps) * np.eye(128)).astype(np.float32)
    return out


def make_in_maps(inputs, NL=4):
    drug_x = np.asarray(inputs["drug_x"], np.float32)
    solv_x = np.asarray(inputs["solvent_x"], np.float32)
    N, FIN = drug_x.shape
    assert N % GS == 0
    NSH = N // GS
    NW = -(-NSH // WIN)
    params = inputs["params"]
    temperature = np.asarray(inputs["temperature"], np.float32)
    B = temperature.shape[0]

    SPLIT = min(32768, N)
    if "split_override" in inputs:
        SPLIT = int(inputs["split_override"])
    enc_data = []
    counts_lo, counts_hi = [], []
    for key_x, key_e, key_b, key_p in (
        ("drug_x", "drug_edge_index", "drug_batch", "drug"),
        ("solvent_x", "solvent_edge_index", "solvent_batch", "solvent"),
    ):
        x = np.asarray(inputs[key_x], np.float32)
        shards = _shard_edges(np.asarray(inputs[key_e]), N, NSH)
        batch = np.asarray(inputs[key_b], np.int64)
        enc_data.append((x, shards, batch, params[key_p]))
        for ss, dd in shards:
            clo, chi = _window_stream_counts(ss, dd, NSH, SPLIT)
            counts_lo.append(clo)
            counts_hi.append(chi)
    T_lo = [int(max(-(-counts_lo[i][w] // 128) for i in range(len(counts_lo))))
            for w in range(NW)]
    T_hi = [int(max(-(-counts_hi[i][w] // 128) for i in range(len(counts_hi))))
            for w in range(NW)]

    io128 = np.tile(np.arange(WIN, dtype=np.float32)[None, :], (128, 1))
    iog = np.tile(np.arange(B, dtype=np.float32)[None, :], (128, 1))
    idn = np.eye(128, dtype=np.float32)
    ones = np.ones((128, 1), np.float32)
    trow = np.ascontiguousarray(temperature.reshape(1, B))

    hp = params["pred"]
    tp = params["temp"]
    pw1 = np.asarray(hp["w1"], np.float32)
    head = {
        "pw1a": pw1[:128], "pw1b": pw1[128:256], "pw1c": pw1[256:288],
        "pb1": np.asarray(hp["b1"], np.float32).reshape(128, 1),
        "pw2": np.asarray(hp["w2"], np.float32),
        "pb2": np.asarray(hp["b2"], np.float32).reshape(64, 1),
        "pw3": np.asarray(hp["w3"], np.float32),
        "pb3": np.asarray(hp["b3"], np.float32).reshape(1, 1),
        "tw1": np.asarray(tp["w1"], np.float32),
        "tb1": np.asarray(tp["b1"], np.float32).reshape(32, 1),
        "tw2": np.asarray(tp["w2"], np.float32),
        "tb2": np.asarray(tp["b2"], np.float32).reshape(32, 1),
    }

    in_maps = []
    for c in range(NCORES):
        e = c // GS
        s = c % GS
        x, shards, batch, enc_p = enc_data[e]
        src_a, dst_a = _build_edge_arrays(shards[s][0], shards[s][1], NSH,
                                          T_lo, T_hi, SPLIT)
        m = {
            "xin": np.ascontiguousarray(x[s * NSH:(s + 1) * NSH].T),
            "srcs": src_a,
            "dsts": dst_a,
            "batf": _batch_array(batch, NSH, s),
            "io128": io128, "iog": iog, "idn": idn, "ones": ones,
            "trow": trow,
        }
        m.update(_enc_param_maps(enc_p, NL))
        m.update(head)
        in_maps.append(m)

    cfg = {"NSH": NSH, "B": B, "FIN": FIN, "NL": NL,
           "T_lo": T_lo, "T_hi": T_hi, "SPLIT": SPLIT}
    return cfg, in_maps


_PROGRAM_CACHE = {}


def _get_program(cfg):
    key = (cfg["NSH"], cfg["B"], cfg["FIN"], cfg["NL"],
           tuple(cfg["T_lo"]), tuple(cfg["T_hi"]), cfg["SPLIT"])
    if key not in _PROGRAM_CACHE:
        _PROGRAM_CACHE[key] = build_program(cfg)
    return _PROGRAM_CACHE[key]


def kernel(**inputs) -> np.ndarray:
    from concourse.bass_utils import run_bass_kernel_spmd
    cfg, in_maps = make_in_maps(inputs)
    nc = _get_program(cfg)
    res = run_bass_kernel_spmd(nc, in_maps, core_ids=list(range(NCORES)))
    out = np.asarray(res.results[0]["out"], np.float32)
    return out.reshape(-1, 1)


# revision 17
# speedup vs baseline: 1.0606x; 1.0340x over previous
"""Trainium2 Bass kernel for the BigSolDB pretrain model (two GIN encoders +
fusion head), distributed over 8 NeuronCores.

Sharding: cores 0-3 run the drug encoder, cores 4-7 the solvent encoder
(identical SPMD program, different per-core data).  Within each group of 4,
nodes are sharded into 4 contiguous ranges; each core handles the edges whose
destination falls in its range.  Per GIN layer the full node-feature table is
rebuilt with an AllGather so every core can gather arbitrary source rows, and
batch-norm statistics are combined with a small AllReduce.  Pooled graph
embeddings are exchanged across the two groups and the fusion head is computed
redundantly on every core.

The edge aggregation (segment_sum of x[src] by dst) runs as:
  - batched indirect DMA row gathers from the HBM feature table, and
  - PE matmuls against on-chip-built one-hot (edge -> dst slot) matrices,
    accumulating each 128-wide dst window in PSUM.
"""

import numpy as np

import concourse.bass as bass
import concourse.bacc as bacc
import concourse.mybir as mybir
import concourse.tile as tile

F32 = mybir.dt.float32
I32 = mybir.dt.int32
AF = mybir.ActivationFunctionType
ALU = mybir.AluOpType

HID = 128
HID2 = 256
NCORES = 8
GS = 4          # cores per encoder group
WIN = 128       # dst-window width (one-hot matmul N dim)
CH = 512        # node-chunk width for the MLP phases
BN_EPS = 1e-5


# ---------------------------------------------------------------------------
# Host-side sharding / scheduling
# ---------------------------------------------------------------------------

def _shard_edges(edge_index, n_nodes, nsh):
    """Split edges by dst shard; per shard return (src, dst_local) sorted by
    dst_local."""
    src = np.asarray(edge_index[0], dtype=np.int64)
    dst = np.asarray(edge_index[1], dtype=np.int64)
    shards = []
    for s in range(GS):
        lo, hi = s * nsh, (s + 1) * nsh
        m = (dst >= lo) & (dst < hi)
        ss, dd = src[m], dst[m] - lo
        order = np.argsort(dd, kind="stable")
        shards.append((ss[order], dd[order]))
    return shards


def _window_counts(dst_local, nsh):
    nw = -(-nsh // WIN)
    return np.bincount(dst_local // WIN, minlength=nw)


def _window_stream_counts(src, dst_local, nsh, split):
    """Per window: (lo_count, hi_count) by src < split."""
    nw = -(-nsh // WIN)
    w_of_edge = dst_local // WIN
    lo = src < split
    clo = np.bincount(w_of_edge[lo], minlength=nw)
    chi = np.bincount(w_of_edge[~lo], minlength=nw)
    return clo, chi


GW = 2  # windows per gather-group


def _grid_layout(T_lo, T_hi):
    """Grouped grid: per group of GW windows, lo blocks then hi blocks.
    Returns (groups, NT): groups = list of (windows, lo_bases, hi_bases,
    gbase, gtiles)."""
    nw = len(T_lo)
    groups = []
    base = 0
    for g0 in range(0, nw, GW):
        ws = list(range(g0, min(g0 + GW, nw)))
        gbase = base
        lo_bases, hi_bases = {}, {}
        for w in ws:
            lo_bases[w] = base
            base += T_lo[w]
        for w in ws:
            hi_bases[w] = base
            base += T_hi[w]
        groups.append((ws, lo_bases, hi_bases, gbase, base - gbase))
    return groups, base


def _build_edge_arrays(src, dst_local, nsh, T_lo, T_hi, split):
    """Pack edges into the shared grouped (lo/hi) tile grid.

    Returns idx16 [128, NT*8] int16 (dma_gather wrapped layout, replicated
    into the 8 Q7-core stripes) and dst_f [128, NT] float32 (-1 in pad
    slots)."""
    nw = len(T_lo)
    groups, NT = _grid_layout(T_lo, T_hi)
    idx16 = np.zeros((128, NT * 8), dtype=np.int16)
    dst_a = np.full((128, NT), -1.0, dtype=np.float32)
    w_of_edge = dst_local // WIN
    starts = np.searchsorted(w_of_edge, np.arange(nw))
    ends = np.searchsorted(w_of_edge, np.arange(nw) + 1)

    def put(base, Ts, es, ed, w):
        cnt = len(es)
        flat_s = np.zeros(Ts * 128, dtype=np.int16)
        flat_d = np.full(Ts * 128, -1.0, dtype=np.float32)
        flat_s[:cnt] = es.astype(np.int16)
        flat_d[:cnt] = ed - w * WIN
        dst_a[:, base:base + Ts] = flat_d.reshape(Ts, 128).T
        k = np.arange(Ts * 128)
        cols = base * 8 + k // 16
        for c in range(8):
            idx16[16 * c + (k % 16), cols] = flat_s

    for ws, lo_bases, hi_bases, gbase, gtiles in groups:
        for w in ws:
            sl = slice(starts[w], ends[w])
            es, ed = src[sl], dst_local[sl]
            lo_m = es < split
            if T_lo[w]:
                put(lo_bases[w], T_lo[w], es[lo_m], ed[lo_m], w)
            if T_hi[w]:
                put(hi_bases[w], T_hi[w], es[~lo_m] - split, ed[~lo_m], w)
    return idx16, dst_a


def _batch_array(batch, nsh, s):
    """[128, NW] float32 graph ids of this shard's nodes, -1 in pad slots."""
    nw = -(-nsh // WIN)
    out = np.full((128, nw), -1.0, dtype=np.float32)
    vals = np.asarray(batch[s * nsh:(s + 1) * nsh], dtype=np.float32)
    pad = nw * WIN - len(vals)
    if pad:
        vals = np.concatenate([vals, np.full(pad, -1.0, np.float32)])
    out[:, :] = vals.reshape(nw, 128).T
    return out


def _chunks(n, ch):
    return [(c0, min(ch, n - c0)) for c0 in range(0, n, ch)]


# ---------------------------------------------------------------------------
# Program builder (one SPMD program shared by all 8 cores)
# ---------------------------------------------------------------------------

def build_program(cfg):
    NSH = cfg["NSH"]
    N = GS * NSH
    NW = -(-NSH // WIN)
    SPLIT = cfg["SPLIT"]
    T_lo = cfg["T_lo"]
    T_hi = cfg["T_hi"]
    groups, NT = _grid_layout(T_lo, T_hi)
    Tmax = max(1, max(T_lo[w] + T_hi[w] for w in range(NW)))
    GTmax = max(1, max(g[4] for g in groups))
    HAS_HI = N > SPLIT
    B = cfg["B"]
    FIN = cfg["FIN"]
    NL = cfg["NL"]
    GH = -(-B // 128)            # graph-halves (1 if B<=128, else 2)
    chunks = _chunks(NSH, CH)
    NCH = len(chunks)
    inv_n = 1.0 / float(N)
    enc_groups = [list(range(GS)), list(range(GS, 2 * GS))]
    pair_groups = [[c, c + GS] for c in range(GS)]

    nc = bacc.Bacc("TRN2", target_bir_lowering=False, debug=False,
                   num_devices=NCORES)

    def din(name, shape, dtype=F32):
        return nc.dram_tensor(name, shape, dtype, kind="ExternalInput")

    xin = din("xin", [FIN, NSH])
    srcs_d = din("srcs", [128, NT * 8], mybir.dt.int16)
    dsts_d = din("dsts", [128, NT])
    batf_d = din("batf", [128, NW])
    io128_d = din("io128", [128, WIN])
    iog_d = din("iog", [128, B])
    idn_d = din("idn", [128, 128])
    ones_d = din("ones", [128, 1])
    trow_d = din("trow", [1, B])

    embw_d = din("emb_w", [FIN, HID])
    embb_d = din("emb_b", [HID, 1])
    lp = []
    for l in range(NL):
        lp.append({k: din(f"{k}_{l}", shp) for k, shp in [
            ("w1", [HID, HID2]),
            ("g1a", [128, 1]), ("g1b", [128, 1]),
            ("be1a", [128, 1]), ("be1b", [128, 1]),
            ("w2a", [128, HID]), ("w2b", [128, HID]),
            ("g2", [HID, 1]), ("be2", [HID, 1]),
            ("epsi", [128, 128]),
        ]})
    hp = {k: din(k, shp) for k, shp in [
        ("pw1a", [128, 128]), ("pw1b", [128, 128]), ("pw1c", [32, 128]),
        ("pb1", [128, 1]), ("pw2", [128, 64]), ("pb2", [64, 1]),
        ("pw3", [64, 1]), ("pb3", [1, 1]),
        ("tw1", [1, 32]), ("tb1", [32, 1]), ("tw2", [32, 32]), ("tb2", [32, 1]),
    ]}

    out_d = nc.dram_tensor("out", [1, B], F32, kind="ExternalOutput")

    # internal DRAM
    xtab_in = nc.dram_tensor("xtab_in", [NSH, HID], F32)
    x_table = nc.dram_tensor("x_table", [N, HID], F32)
    xtab_hi = (nc.dram_tensor("xtab_hi", [N - SPLIT, HID], F32)
               if HAS_HI else None)
    st_bufs = []
    for l in range(NL):
        st_bufs.append((
            nc.dram_tensor(f"s1i_{l}", [128, 4], F32),
            nc.dram_tensor(f"s1o_{l}", [128, 4], F32),
            nc.dram_tensor(f"s2i_{l}", [128, 2], F32),
            nc.dram_tensor(f"s2o_{l}", [128, 2], F32),
        ))
    GH_ = -(-B // 128)
    pacc_in = nc.dram_tensor("pacc_in", [128, GH_ * 132], F32)
    pacc_out = nc.dram_tensor("pacc_out", [128, GH_ * 132], F32)
    pool_in = nc.dram_tensor("pool_in", [128, B], F32)
    pool_out = nc.dram_tensor("pool_out", [256, B], F32)

    with tile.TileContext(nc) as tc:
        with (
            tc.tile_pool(name="persist", bufs=1) as pp,
            tc.tile_pool(name="wrk", bufs=3) as wk,
            tc.tile_pool(name="gat", bufs=3) as gp,
        ):
            # ---- persistent loads ----
            xT = pp.tile([128, NSH], F32, tag="xT")
            dsts = pp.tile([128, NT], F32, tag="dsts")
            batf = pp.tile([128, NW], F32, tag="batf")
            io128 = pp.tile([128, WIN], F32, tag="io128")
            iog = pp.tile([128, B], F32, tag="iog")
            idn = pp.tile([128, 128], F32, tag="idn")
            onec = pp.tile([128, 1], F32, tag="onec")
            trow = pp.tile([1, B], F32, tag="trow")
            ssum = pp.tile([128, 2 * NCH], F32, tag="ssum")
            ssq = pp.tile([128, 2 * NCH], F32, tag="ssq")
            s2sum = pp.tile([128, NCH], F32, tag="s2sum")
            s2sq = pp.tile([128, NCH], F32, tag="s2sq")

            nc.sync.dma_start(out=dsts[:], in_=dsts_d[:, :])
            nc.sync.dma_start(out=batf[:], in_=batf_d[:, :])
            nc.sync.dma_start(out=io128[:], in_=io128_d[:, :])
            nc.sync.dma_start(out=iog[:], in_=iog_d[:, :])
            nc.sync.dma_start(out=idn[:], in_=idn_d[:, :])
            nc.sync.dma_start(out=onec[:], in_=ones_d[:, :])
            nc.sync.dma_start(out=trow[:], in_=trow_d[:, :])

            embw = pp.tile([FIN, HID], F32, tag="embw")
            embb = pp.tile([HID, 1], F32, tag="embb")
            nc.sync.dma_start(out=embw[:], in_=embw_d[:, :])
            nc.sync.dma_start(out=embb[:], in_=embb_d[:, :])
            lt = []
            for l in range(NL):
                d = {}
                for k, h in lp[l].items():
                    d[k] = pp.tile(list(h.shape), F32, tag=f"{k}_{l}",
                                   name=f"{k}_{l}_t")
                    nc.sync.dma_start(out=d[k][:], in_=h[:, :])
                lt.append(d)
            ht = {}
            for k, h in hp.items():
                ht[k] = pp.tile(list(h.shape), F32, tag=k, name=f"{k}_t")
                nc.sync.dma_start(out=ht[k][:], in_=h[:, :])

            with (
                tc.tile_pool(name="pq", bufs=3, space="PSUM") as pqp,
                tc.tile_pool(name="pmlp", bufs=2, space="PSUM") as pmlp,
            ):
                # ---- embedding: xT = emb_w.T @ xin (+ emb_b) ----
                for ci, (c0, cw) in enumerate(chunks):
                    xc = wk.tile([FIN, CH], F32, tag="xc")
                    nc.sync.dma_start(out=xc[:, :cw], in_=xin[:, c0:c0 + cw])
                    pe = pmlp.tile([128, CH], F32, space="PSUM", tag="pm0")
                    nc.tensor.matmul(out=pe[:, :cw], lhsT=embw[:],
                                     rhs=xc[:, :cw], start=True, stop=True)
                    nc.vector.tensor_scalar_add(out=xT[:, c0:c0 + cw],
                                                in0=pe[:, :cw],
                                                scalar1=embb[:, :1])

                # ---- GIN layers ----
                for l in range(NL):
                    d = lt[l]
                    s1i, s1o, s2i, s2o = st_bufs[l]

                    # 1) rebuild the gather table: x_table <- AllGather(x_loc)
                    for w in range(NW):
                        w0 = w * WIN
                        ww = min(WIN, NSH - w0)
                        tp = pqp.tile([128, 128], F32, space="PSUM", tag="pq")
                        nc.tensor.transpose(out=tp[:ww, :],
                                            in_=xT[:, w0:w0 + ww],
                                            identity=idn[:])
                        ts = wk.tile([128, 128], F32, tag="tts")
                        nc.scalar.copy(out=ts[:ww, :], in_=tp[:ww, :])
                        nc.sync.dma_start(out=xtab_in[w0:w0 + ww, :],
                                          in_=ts[:ww, :])
                    nc.gpsimd.collective_compute(
                        "AllGather", ALU.bypass, replica_groups=enc_groups,
                        ins=[xtab_in.ap().opt()], outs=[x_table.ap().opt()])
                    if HAS_HI:
                        nc.sync.dma_start(out=xtab_hi[:, :],
                                          in_=x_table[SPLIT:, :])

                    # phase-3 emitter: mm1 + BN1 stats for one chunk
                    def emit_p3(ci, d=d):
                        c0, cw = chunks[ci]
                        for hf in range(2):
                            pm = pmlp.tile([128, CH], F32, space="PSUM",
                                           tag=f"pm{hf}", name=f"pm_{hf}_{ci}")
                            nc.tensor.matmul(
                                out=pm[:, :cw],
                                lhsT=d["w1"][:, hf * 128:(hf + 1) * 128],
                                rhs=xT[:, c0:c0 + cw], start=True, stop=True)
                            sc1 = wk.tile([128, CH], F32, tag="sc1",
                                          name=f"sc1_{ci}_{hf}")
                            nc.scalar.activation(
                                out=sc1[:, :cw], in_=pm[:, :cw], func=AF.Copy,
                                accum_out=ssum[:, hf * NCH + ci:
                                               hf * NCH + ci + 1])
                            sc2 = wk.tile([128, CH], F32, tag="sc2",
                                          name=f"sc2_{ci}_{hf}")
                            nc.scalar.activation(
                                out=sc2[:, :cw], in_=pm[:, :cw],
                                func=AF.Square,
                                accum_out=ssq[:, hf * NCH + ci:
                                              hf * NCH + ci + 1])

                    next_chunk = 0

                    # 2) aggregate + h = (1+eps)*x + agg   (in place on xT)
                    for gi, (ws, lo_bases, hi_bases, gbase, gtiles) in \
                            enumerate(groups):
                        GTl = sum(T_lo[w] for w in ws)
                        GTh = sum(T_hi[w] for w in ws)
                        gt = None
                        if gtiles > 0:
                            idxw = gp.tile([128, GTmax * 8], mybir.dt.int16,
                                           tag="idxw", bufs=2)
                            nc.sync.dma_start(
                                out=idxw[:, :gtiles * 8],
                                in_=srcs_d[:, gbase * 8:(gbase + gtiles) * 8])
                            gt = gp.tile([128, GTmax * HID], F32, tag="gt",
                                         bufs=2)
                            if GTl > 0:
                                nc.gpsimd.dma_gather(
                                    out_ap=gt[:, :GTl * HID].rearrange(
                                        "p (t d) -> p t d", d=HID),
                                    in_ap=x_table[:, :],
                                    idxs_ap=idxw[:, :GTl * 8],
                                    num_idxs=GTl * 128,
                                    num_idxs_reg=GTl * 128,
                                    elem_size=HID,
                                    single_packet=False)
                            if GTh > 0:
                                nc.gpsimd.dma_gather(
                                    out_ap=gt[:, GTl * HID:gtiles * HID
                                              ].rearrange(
                                        "p (t d) -> p t d", d=HID),
                                    in_ap=xtab_hi[:, :],
                                    idxs_ap=idxw[:, GTl * 8:gtiles * 8],
                                    num_idxs=GTh * 128,
                                    num_idxs_reg=GTh * 128,
                                    elem_size=HID,
                                    single_packet=False)
                        for w in ws:
                            Tl, Th = T_lo[w], T_hi[w]
                            Tw = Tl + Th
                            w0 = w * WIN
                            ww = min(WIN, NSH - w0)
                            pag = pqp.tile([128, WIN], F32, space="PSUM",
                                           tag="pq")
                            nc.tensor.matmul(out=pag[:, :ww],
                                             lhsT=d["epsi"][:],
                                             rhs=xT[:, w0:w0 + ww],
                                             start=True, stop=(Tw == 0))
                            if Tw > 0:
                                mt = gp.tile([128, Tmax * WIN], F32, tag="mt",
                                             bufs=2)
                                for si, (sb, Ts) in enumerate(
                                        ((lo_bases[w], Tl), (hi_bases[w], Th))):
                                    if Ts == 0:
                                        continue
                                    moff = 0 if si == 0 else Tl * ww
                                    din_ap = dsts[:, sb:sb + Ts]
                                    in0 = bass.AP(
                                        din_ap.tensor, din_ap.offset,
                                        [din_ap.ap[0], [din_ap.ap[1][0], Ts],
                                         [0, ww]])
                                    io_ap = io128[:, :ww]
                                    in1 = bass.AP(
                                        io_ap.tensor, io_ap.offset,
                                        [io_ap.ap[0], [0, Ts], [1, ww]])
                                    nc.vector.tensor_tensor(
                                        out=mt[:, moff:moff + Ts * ww
                                               ].rearrange(
                                            "p (t w) -> p t w", t=Ts),
                                        in0=in0, in1=in1, op=ALU.is_equal)
                                for t in range(Tw):
                                    grid_col = (lo_bases[w] + t if t < Tl
                                                else hi_bases[w] + (t - Tl))
                                    nc.tensor.matmul(
                                        out=pag[:, :ww],
                                        lhsT=gt[:, (grid_col - gbase) * HID:
                                                (grid_col - gbase + 1) * HID],
                                        rhs=mt[:, t * ww:(t + 1) * ww],
                                        start=False, stop=(t == Tw - 1))
                            nc.vector.tensor_copy(out=xT[:, w0:w0 + ww],
                                                  in_=pag[:, :ww])
                        last_win = min(2 * gi + 1, NW - 1)
                        while (next_chunk < NCH and
                               (min((chunks[next_chunk][0] +
                                     chunks[next_chunk][1] - 1) // WIN,
                                    NW - 1) <= last_win)):
                            emit_p3(next_chunk)
                            next_chunk += 1

                    while next_chunk < NCH:
                        emit_p3(next_chunk)
                        next_chunk += 1

                    st = wk.tile([128, 4], F32, tag="st")
                    for hf in range(2):
                        nc.vector.tensor_reduce(
                            out=st[:, hf:hf + 1],
                            in_=ssum[:, hf * NCH:(hf + 1) * NCH],
                            axis=mybir.AxisListType.X, op=ALU.add)
                        nc.vector.tensor_reduce(
                            out=st[:, 2 + hf:3 + hf],
                            in_=ssq[:, hf * NCH:(hf + 1) * NCH],
                            axis=mybir.AxisListType.X, op=ALU.add)
                    nc.sync.dma_start(out=s1i[:, :], in_=st[:])
                    nc.gpsimd.collective_compute(
                        "AllReduce", ALU.add, replica_groups=enc_groups,
                        ins=[s1i.ap().opt()], outs=[s1o.ap().opt()])
                    sr = wk.tile([128, 4], F32, tag="sr")
                    nc.sync.dma_start(out=sr[:], in_=s1o[:, :])

                    sca, sha = [], []
                    for hf in range(2):
                        g_t = d["g1a"] if hf == 0 else d["g1b"]
                        be_t = d["be1a"] if hf == 0 else d["be1b"]
                        mean = wk.tile([128, 1], F32, tag=f"mean{hf}")
                        nc.vector.tensor_scalar_mul(
                            out=mean[:], in0=sr[:, hf:hf + 1], scalar1=inv_n)
                        var = wk.tile([128, 1], F32, tag=f"var{hf}")
                        # var = sq/n - mean^2 ; then + eps
                        nc.vector.tensor_scalar_mul(
                            out=var[:], in0=sr[:, 2 + hf:3 + hf], scalar1=inv_n)
                        m2 = wk.tile([128, 1], F32, tag=f"m2{hf}")
                        nc.vector.tensor_tensor(out=m2[:], in0=mean[:],
                                                in1=mean[:], op=ALU.mult)
                        nc.vector.tensor_tensor(out=var[:], in0=var[:],
                                                in1=m2[:], op=ALU.subtract)
                        nc.vector.tensor_scalar_add(out=var[:], in0=var[:],
                                                    scalar1=BN_EPS)
                        nc.vector.reciprocal(out=var[:], in_=var[:])
                        inv = wk.tile([128, 1], F32, tag=f"inv{hf}")
                        nc.scalar.activation(out=inv[:], in_=var[:],
                                             func=AF.Sqrt)
                        sc = wk.tile([128, 1], F32, tag=f"sc{hf}")
                        nc.vector.tensor_tensor(out=sc[:], in0=g_t[:],
                                                in1=inv[:], op=ALU.mult)
                        sh = wk.tile([128, 1], F32, tag=f"sh{hf}")
                        nc.vector.tensor_tensor(out=sh[:], in0=mean[:],
                                                in1=sc[:], op=ALU.mult)
                        nc.vector.tensor_tensor(out=sh[:], in0=be_t[:],
                                                in1=sh[:], op=ALU.subtract)
                        sca.append(sc)
                        sha.append(sh)

                    # 4) BN1 apply + relu + matmul2 + BN2 stats (recompute mm1)
                    for ci, (c0, cw) in enumerate(chunks):
                        hrs = []
                        for hf in range(2):
                            pm = pmlp.tile([128, CH], F32, space="PSUM",
                                           tag=f"pm{hf}")
                            nc.tensor.matmul(
                                out=pm[:, :cw],
                                lhsT=d["w1"][:, hf * 128:(hf + 1) * 128],
                                rhs=xT[:, c0:c0 + cw], start=True, stop=True)
                            hr = wk.tile([128, CH], F32, tag=f"hr{hf}")
                            nc.scalar.activation(
                                out=hr[:, :cw], in_=pm[:, :cw], func=AF.Relu,
                                bias=sha[hf][:, :1], scale=sca[hf][:, :1])
                            hrs.append(hr)
                        pm2 = pmlp.tile([128, CH], F32, space="PSUM", tag="pm2",
                                        bufs=1)
                        nc.tensor.matmul(out=pm2[:, :cw], lhsT=d["w2a"][:],
                                         rhs=hrs[0][:, :cw], start=True,
                                         stop=False)
                        nc.tensor.matmul(out=pm2[:, :cw], lhsT=d["w2b"][:],
                                         rhs=hrs[1][:, :cw], start=False,
                                         stop=True)
                        nc.scalar.activation(
                            out=xT[:, c0:c0 + cw], in_=pm2[:, :cw],
                            func=AF.Copy,
                            accum_out=s2sum[:, ci:ci + 1])
                        sc2 = wk.tile([128, CH], F32, tag="sc2")
                        nc.scalar.activation(
                            out=sc2[:, :cw], in_=pm2[:, :cw], func=AF.Square,
                            accum_out=s2sq[:, ci:ci + 1])

                    st2 = wk.tile([128, 2], F32, tag="st2")
                    nc.vector.tensor_reduce(out=st2[:, 0:1], in_=s2sum[:, :],
                                            axis=mybir.AxisListType.X,
                                            op=ALU.add)
                    nc.vector.tensor_reduce(out=st2[:, 1:2], in_=s2sq[:, :],
                                            axis=mybir.AxisListType.X,
                                            op=ALU.add)
                    nc.sync.dma_start(out=s2i[:, :], in_=st2[:])
                    nc.gpsimd.collective_compute(
                        "AllReduce", ALU.add, replica_groups=enc_groups,
                        ins=[s2i.ap().opt()], outs=[s2o.ap().opt()])
                    sr2 = wk.tile([128, 2], F32, tag="sr2")
                    nc.sync.dma_start(out=sr2[:], in_=s2o[:, :])

                    mean = wk.tile([128, 1], F32, tag="mean2")
                    nc.vector.tensor_scalar_mul(out=mean[:], in0=sr2[:, 0:1],
                                                scalar1=inv_n)
                    var = wk.tile([128, 1], F32, tag="var2")
                    nc.vector.tensor_scalar_mul(out=var[:], in0=sr2[:, 1:2],
                                                scalar1=inv_n)
                    m2 = wk.tile([128, 1], F32, tag="m22")
                    nc.vector.tensor_tensor(out=m2[:], in0=mean[:], in1=mean[:],
                                            op=ALU.mult)
                    nc.vector.tensor_tensor(out=var[:], in0=var[:], in1=m2[:],
                                            op=ALU.subtract)
                    nc.vector.tensor_scalar_add(out=var[:], in0=var[:],
                                                scalar1=BN_EPS)
                    nc.vector.reciprocal(out=var[:], in_=var[:])
                    inv = wk.tile([128, 1], F32, tag="inv2")
                    nc.scalar.activation(out=inv[:], in_=var[:], func=AF.Sqrt)
                    sc2t = wk.tile([128, 1], F32, tag="sc2t")
                    nc.vector.tensor_tensor(out=sc2t[:], in0=d["g2"][:],
                                            in1=inv[:], op=ALU.mult)
                    sh2t = wk.tile([128, 1], F32, tag="sh2t")
                    nc.vector.tensor_tensor(out=sh2t[:], in0=mean[:],
                                            in1=sc2t[:], op=ALU.mult)
                    nc.vector.tensor_tensor(out=sh2t[:], in0=d["be2"][:],
                                            in1=sh2t[:], op=ALU.subtract)

                    for ci, (c0, cw) in enumerate(chunks):
                        nc.scalar.activation(out=xT[:, c0:c0 + cw],
                                             in_=xT[:, c0:c0 + cw],
                                             func=AF.Relu, bias=sh2t[:, :1],
                                             scale=sc2t[:, :1])

            # ---- pooling + head (fresh PSUM pools) ----
            with (
                tc.tile_pool(name="ppool", bufs=1, space="PSUM") as ppl,
                tc.tile_pool(name="ptr", bufs=2, space="PSUM") as ptr,
                tc.tile_pool(name="phd", bufs=2, space="PSUM") as phd,
            ):
                # tags: plg0/plg1 (1 bank each), ptp (2), ph (2) => 6 banks
                plg = [ppl.tile([128, 132], F32, space="PSUM", tag=f"plg{g}",
                                name=f"plg{g}")
                       for g in range(GH)]
                for w in range(NW):
                    w0 = w * WIN
                    ww = min(WIN, NSH - w0)
                    tp = ptr.tile([128, 128], F32, space="PSUM", tag="ptp")
                    nc.tensor.transpose(out=tp[:ww, :], in_=xT[:, w0:w0 + ww],
                                        identity=idn[:])
                    xw = wk.tile([128, 132], F32, tag="xw")
                    if ww < 128:
                        nc.vector.memset(xw[:], 0.0)
                    nc.scalar.copy(out=xw[:ww, :128], in_=tp[:ww, :])
                    nc.vector.tensor_copy(out=xw[:, 128:129], in_=onec[:, :1])
                    bt = wk.tile([128, B], F32, tag="bt")
                    b_ap = batf[:, w:w + 1]
                    nc.vector.tensor_tensor(
                        out=bt[:, :B],
                        in0=bass.AP(b_ap.tensor, b_ap.offset,
                                    [b_ap.ap[0], [0, B]]),
                        in1=iog[:, :B], op=ALU.is_equal)
                    for g in range(GH):
                        gw = min(128, B - g * 128)
                        nc.tensor.matmul(out=plg[g][:gw, 0:129],
                                         lhsT=bt[:, g * 128:g * 128 + gw],
                                         rhs=xw[:, :129], start=(w == 0),
                                         stop=(w == NW - 1))

                # combine shard-partial pooled sums/counts across the group
                pacc = wk.tile([128, GH * 132], F32, tag="pacc")
                nc.vector.memset(pacc[:], 0.0)
                for g in range(GH):
                    gw = min(128, B - g * 128)
                    nc.scalar.copy(out=pacc[:gw, g * 132:g * 132 + 129],
                                   in_=plg[g][:gw, :129])
                nc.sync.dma_start(out=pacc_in[:, :], in_=pacc[:])
                nc.gpsimd.collective_compute(
                    "AllReduce", ALU.add, replica_groups=enc_groups,
                    ins=[pacc_in.ap().opt()], outs=[pacc_out.ap().opt()])
                pr = wk.tile([128, GH * 132], F32, tag="pr")
                nc.sync.dma_start(out=pr[:], in_=pacc_out[:, :])

                embT = wk.tile([128, B], F32, tag="embT")
                for g in range(GH):
                    gw = min(128, B - g * 128)
                    invc = wk.tile([128, 1], F32, tag="invc")
                    nc.vector.tensor_scalar_max(
                        out=invc[:gw, :],
                        in0=pr[:gw, g * 132 + 128:g * 132 + 129],
                        scalar1=1.0)
                    nc.vector.reciprocal(out=invc[:gw, :], in_=invc[:gw, :])
                    pgs = wk.tile([128, 128], F32, tag="pgs")
                    nc.scalar.activation(out=pgs[:gw, :],
                                         in_=pr[:gw, g * 132:g * 132 + 128],
                                         func=AF.Copy, scale=invc[:gw, :1])
                    tpp = ptr.tile([128, 128], F32, space="PSUM", tag="ptp")
                    nc.tensor.transpose(out=tpp[:, :gw], in_=pgs[:gw, :],
                                        identity=idn[:gw, :gw])
                    nc.scalar.copy(out=embT[:, g * 128:g * 128 + gw],
                                   in_=tpp[:, :gw])

                nc.sync.dma_start(out=pool_in[:, :], in_=embT[:])
                nc.gpsimd.collective_compute(
                    "AllGather", ALU.bypass, replica_groups=pair_groups,
                    ins=[pool_in.ap().opt()], outs=[pool_out.ap().opt()])
                demb = wk.tile([128, B], F32, tag="demb")
                semb = wk.tile([128, B], F32, tag="semb")
                nc.sync.dma_start(out=demb[:], in_=pool_out[0:128, :])
                nc.sync.dma_start(out=semb[:], in_=pool_out[128:256, :])

                # temperature MLP
                ptm = phd.tile([32, B], F32, space="PSUM", tag="ph")
                nc.tensor.matmul(out=ptm[:], lhsT=ht["tw1"][:], rhs=trow[:],
                                 start=True, stop=True)
                t1 = wk.tile([32, B], F32, tag="t1")
                nc.scalar.activation(out=t1[:], in_=ptm[:], func=AF.Relu,
                                     bias=ht["tb1"][:, :1])
                ptm2 = phd.tile([32, B], F32, space="PSUM", tag="ph")
                nc.tensor.matmul(out=ptm2[:], lhsT=ht["tw2"][:], rhs=t1[:],
                                 start=True, stop=True)
                t2 = wk.tile([32, B], F32, tag="t2")
                nc.vector.tensor_scalar_add(out=t2[:], in0=ptm2[:],
                                            scalar1=ht["tb2"][:, :1])

                # prediction head
                ph1 = phd.tile([128, B], F32, space="PSUM", tag="ph")
                nc.tensor.matmul(out=ph1[:], lhsT=ht["pw1a"][:], rhs=demb[:],
                                 start=True, stop=False)
                nc.tensor.matmul(out=ph1[:], lhsT=ht["pw1b"][:], rhs=semb[:],
                                 start=False, stop=False)
                nc.tensor.matmul(out=ph1[:], lhsT=ht["pw1c"][:], rhs=t2[:],
                                 start=False, stop=True)
                h1s = wk.tile([128, B], F32, tag="h1s")
                nc.scalar.activation(out=h1s[:], in_=ph1[:], func=AF.Relu,
                                     bias=ht["pb1"][:, :1])
                ph2 = phd.tile([64, B], F32, space="PSUM", tag="ph")
                nc.tensor.matmul(out=ph2[:], lhsT=ht["pw2"][:], rhs=h1s[:],
                                 start=True, stop=True)
                h2s = wk.tile([64, B], F32, tag="h2s")
                nc.scalar.activation(out=h2s[:], in_=ph2[:], func=AF.Relu,
                                     bias=ht["pb2"][:, :1])
                ph3 = phd.tile([1, B], F32, space="PSUM", tag="ph")
                nc.tensor.matmul(out=ph3[:], lhsT=ht["pw3"][:], rhs=h2s[:],
                                 start=True, stop=True)
                oT = wk.tile([1, B], F32, tag="oT")
                nc.vector.tensor_scalar_add(out=oT[:], in0=ph3[:],
                                            scalar1=ht["pb3"][:, :1])
                nc.sync.dma_start(out=out_d[:, :], in_=oT[:])

    nc.compile()
    return nc


# ---------------------------------------------------------------------------
# Input packing
# ---------------------------------------------------------------------------

def _enc_param_maps(enc, NL):
    """Per-encoder named parameter arrays for the device program."""
    out = {
        "emb_w": np.asarray(enc["emb_w"], np.float32),
        "emb_b": np.asarray(enc["emb_b"], np.float32).reshape(HID, 1),
    }
    for l in range(NL):
        p = enc["layers"][l]
        w1 = np.asarray(p["w1"], np.float32)
        g1 = np.asarray(p["g1"], np.float32)
        be1 = np.asarray(p["be1"], np.float32)
        w2 = np.asarray(p["w2"], np.float32)
        eps = float(np.asarray(p["eps"]))
        out[f"w1_{l}"] = w1
        out[f"g1a_{l}"] = g1[:128].reshape(128, 1)
        out[f"g1b_{l}"] = g1[128:].reshape(128, 1)
        out[f"be1a_{l}"] = be1[:128].reshape(128, 1)
        out[f"be1b_{l}"] = be1[128:].reshape(128, 1)
        out[f"w2a_{l}"] = w2[:128]
        out[f"w2b_{l}"] = w2[128:]
        out[f"g2_{l}"] = np.asarray(p["g2"], np.float32).reshape(HID, 1)
        out[f"be2_{l}"] = np.asarray(p["be2"], np.float32).reshape(HID, 1)
        out[f"epsi_{l}"] = ((1.0 + eps) * np.eye(128)).astype(np.float32)
    return out


def make_in_maps(inputs, NL=4):
    drug_x = np.asarray(inputs["drug_x"], np.float32)
    solv_x = np.asarray(inputs["solvent_x"], np.float32)
    N, FIN = drug_x.shape
    assert N % GS == 0
    NSH = N // GS
    NW = -(-NSH // WIN)
    params = inputs["params"]
    temperature = np.asarray(inputs["temperature"], np.float32)
    B = temperature.shape[0]

    SPLIT = min(32768, N)
    if "split_override" in inputs:
        SPLIT = int(inputs["split_override"])
    enc_data = []
    counts_lo, counts_hi = [], []
    for key_x, key_e, key_b, key_p in (
        ("drug_x", "drug_edge_index", "drug_batch", "drug"),
        ("solvent_x", "solvent_edge_index", "solvent_batch", "solvent"),
    ):
        x = np.asarray(inputs[key_x], np.float32)
        shards = _shard_edges(np.asarray(inputs[key_e]), N, NSH)
        batch = np.asarray(inputs[key_b], np.int64)
        enc_data.append((x, shards, batch, params[key_p]))
        for ss, dd in shards:
            clo, chi = _window_stream_counts(ss, dd, NSH, SPLIT)
            counts_lo.append(clo)
            counts_hi.append(chi)
    T_lo = [int(max(-(-counts_lo[i][w] // 128) for i in range(len(counts_lo))))
            for w in range(NW)]
    T_hi = [int(max(-(-counts_hi[i][w] // 128) for i in range(len(counts_hi))))
            for w in range(NW)]

    io128 = np.tile(np.arange(WIN, dtype=np.float32)[None, :], (128, 1))
    iog = np.tile(np.arange(B, dtype=np.float32)[None, :], (128, 1))
    idn = np.eye(128, dtype=np.float32)
    ones = np.ones((128, 1), np.float32)
    trow = np.ascontiguousarray(temperature.reshape(1, B))

    hp = params["pred"]
    tp = params["temp"]
    pw1 = np.asarray(hp["w1"], np.float32)
    head = {
        "pw1a": pw1[:128], "pw1b": pw1[128:256], "pw1c": pw1[256:288],
        "pb1": np.asarray(hp["b1"], np.float32).reshape(128, 1),
        "pw2": np.asarray(hp["w2"], np.float32),
        "pb2": np.asarray(hp["b2"], np.float32).reshape(64, 1),
        "pw3": np.asarray(hp["w3"], np.float32),
        "pb3": np.asarray(hp["b3"], np.float32).reshape(1, 1),
        "tw1": np.asarray(tp["w1"], np.float32),
        "tb1": np.asarray(tp["b1"], np.float32).reshape(32, 1),
        "tw2": np.asarray(tp["w2"], np.float32),
        "tb2": np.asarray(tp["b2"], np.float32).reshape(32, 1),
    }

    in_maps = []
    for c in range(NCORES):
        e = c // GS
        s = c % GS
        x, shards, batch, enc_p = enc_data[e]
        src_a, dst_a = _build_edge_arrays(shards[s][0], shards[s][1], NSH,
                                          T_lo, T_hi, SPLIT)
        m = {
            "xin": np.ascontiguousarray(x[s * NSH:(s + 1) * NSH].T),
            "srcs": src_a,
            "dsts": dst_a,
            "batf": _batch_array(batch, NSH, s),
            "io128": io128, "iog": iog, "idn": idn, "ones": ones,
            "trow": trow,
        }
        m.update(_enc_param_maps(enc_p, NL))
        m.update(head)
        in_maps.append(m)

    cfg = {"NSH": NSH, "B": B, "FIN": FIN, "NL": NL,
           "T_lo": T_lo, "T_hi": T_hi, "SPLIT": SPLIT}
    return cfg, in_maps


_PROGRAM_CACHE = {}


def _get_program(cfg):
    key = (cfg["NSH"], cfg["B"], cfg["FIN"], cfg["NL"],
           tuple(cfg["T_lo"]), tuple(cfg["T_hi"]), cfg["SPLIT"])
    if key not in _PROGRAM_CACHE:
        _PROGRAM_CACHE[key] = build_program(cfg)
    return _PROGRAM_CACHE[key]


def kernel(**inputs) -> np.ndarray:
    from concourse.bass_utils import run_bass_kernel_spmd
    cfg, in_maps = make_in_maps(inputs)
    nc = _get_program(cfg)
    res = run_bass_kernel_spmd(nc, in_maps, core_ids=list(range(NCORES)))
    out = np.asarray(res.results[0]["out"], np.float32)
    return out.reshape(-1, 1)


# revision 18
# speedup vs baseline: 1.0664x; 1.0055x over previous
"""Trainium2 Bass kernel for the BigSolDB pretrain model (two GIN encoders +
fusion head), distributed over 8 NeuronCores.

Sharding: cores 0-3 run the drug encoder, cores 4-7 the solvent encoder
(identical SPMD program, different per-core data).  Within each group of 4,
nodes are sharded into 4 contiguous ranges; each core handles the edges whose
destination falls in its range.  Per GIN layer the full node-feature table is
rebuilt with an AllGather so every core can gather arbitrary source rows, and
batch-norm statistics are combined with a small AllReduce.  Pooled graph
embeddings are exchanged across the two groups and the fusion head is computed
redundantly on every core.

The edge aggregation (segment_sum of x[src] by dst) runs as:
  - batched indirect DMA row gathers from the HBM feature table, and
  - PE matmuls against on-chip-built one-hot (edge -> dst slot) matrices,
    accumulating each 128-wide dst window in PSUM.
"""

import numpy as np

import concourse.bass as bass
import concourse.bacc as bacc
import concourse.mybir as mybir
import concourse.tile as tile

F32 = mybir.dt.float32
I32 = mybir.dt.int32
AF = mybir.ActivationFunctionType
ALU = mybir.AluOpType

HID = 128
HID2 = 256
NCORES = 8
GS = 4          # cores per encoder group
WIN = 128       # dst-window width (one-hot matmul N dim)
CH = 512        # node-chunk width for the MLP phases
BN_EPS = 1e-5


# ---------------------------------------------------------------------------
# Host-side sharding / scheduling
# ---------------------------------------------------------------------------

def _shard_edges(edge_index, n_nodes, nsh):
    """Split edges by dst shard; per shard return (src, dst_local) sorted by
    dst_local."""
    src = np.asarray(edge_index[0], dtype=np.int64)
    dst = np.asarray(edge_index[1], dtype=np.int64)
    shards = []
    for s in range(GS):
        lo, hi = s * nsh, (s + 1) * nsh
        m = (dst >= lo) & (dst < hi)
        ss, dd = src[m], dst[m] - lo
        order = np.argsort(dd, kind="stable")
        shards.append((ss[order], dd[order]))
    return shards


def _window_counts(dst_local, nsh):
    nw = -(-nsh // WIN)
    return np.bincount(dst_local // WIN, minlength=nw)


def _window_stream_counts(src, dst_local, nsh, split):
    """Per window: (lo_count, hi_count) by src < split."""
    nw = -(-nsh // WIN)
    w_of_edge = dst_local // WIN
    lo = src < split
    clo = np.bincount(w_of_edge[lo], minlength=nw)
    chi = np.bincount(w_of_edge[~lo], minlength=nw)
    return clo, chi


GW = 2  # windows per gather-group


def _grid_layout(T_lo, T_hi):
    """Grouped grid: per group of GW windows, lo blocks then hi blocks.
    Returns (groups, NT): groups = list of (windows, lo_bases, hi_bases,
    gbase, gtiles)."""
    nw = len(T_lo)
    groups = []
    base = 0
    for g0 in range(0, nw, GW):
        ws = list(range(g0, min(g0 + GW, nw)))
        gbase = base
        lo_bases, hi_bases = {}, {}
        for w in ws:
            lo_bases[w] = base
            base += T_lo[w]
        for w in ws:
            hi_bases[w] = base
            base += T_hi[w]
        groups.append((ws, lo_bases, hi_bases, gbase, base - gbase))
    return groups, base


def _build_edge_arrays(src, dst_local, nsh, T_lo, T_hi, split):
    """Pack edges into the shared grouped (lo/hi) tile grid.

    Returns idx16 [128, NT*8] int16 (dma_gather wrapped layout, replicated
    into the 8 Q7-core stripes) and dst_f [128, NT] float32 (-1 in pad
    slots)."""
    nw = len(T_lo)
    groups, NT = _grid_layout(T_lo, T_hi)
    idx16 = np.zeros((128, NT * 8), dtype=np.int16)
    dst_a = np.full((128, NT), -1.0, dtype=np.float32)
    w_of_edge = dst_local // WIN
    starts = np.searchsorted(w_of_edge, np.arange(nw))
    ends = np.searchsorted(w_of_edge, np.arange(nw) + 1)

    def put(base, Ts, es, ed, w):
        cnt = len(es)
        flat_s = np.zeros(Ts * 128, dtype=np.int16)
        flat_d = np.full(Ts * 128, -1.0, dtype=np.float32)
        flat_s[:cnt] = es.astype(np.int16)
        flat_d[:cnt] = ed - w * WIN
        dst_a[:, base:base + Ts] = flat_d.reshape(Ts, 128).T
        k = np.arange(Ts * 128)
        cols = base * 8 + k // 16
        for c in range(8):
            idx16[16 * c + (k % 16), cols] = flat_s

    for ws, lo_bases, hi_bases, gbase, gtiles in groups:
        for w in ws:
            sl = slice(starts[w], ends[w])
            es, ed = src[sl], dst_local[sl]
            lo_m = es < split
            if T_lo[w]:
                put(lo_bases[w], T_lo[w], es[lo_m], ed[lo_m], w)
            if T_hi[w]:
                put(hi_bases[w], T_hi[w], es[~lo_m] - split, ed[~lo_m], w)
    return idx16, dst_a


def _batch_array(batch, nsh, s):
    """[128, NW] float32 graph ids of this shard's nodes, -1 in pad slots."""
    nw = -(-nsh // WIN)
    out = np.full((128, nw), -1.0, dtype=np.float32)
    vals = np.asarray(batch[s * nsh:(s + 1) * nsh], dtype=np.float32)
    pad = nw * WIN - len(vals)
    if pad:
        vals = np.concatenate([vals, np.full(pad, -1.0, np.float32)])
    out[:, :] = vals.reshape(nw, 128).T
    return out


def _chunks(n, ch):
    return [(c0, min(ch, n - c0)) for c0 in range(0, n, ch)]


# ---------------------------------------------------------------------------
# Program builder (one SPMD program shared by all 8 cores)
# ---------------------------------------------------------------------------

def build_program(cfg):
    NSH = cfg["NSH"]
    N = GS * NSH
    NW = -(-NSH // WIN)
    SPLIT = cfg["SPLIT"]
    T_lo = cfg["T_lo"]
    T_hi = cfg["T_hi"]
    groups, NT = _grid_layout(T_lo, T_hi)
    Tmax = max(1, max(T_lo[w] + T_hi[w] for w in range(NW)))
    GTmax = max(1, max(g[4] for g in groups))
    HAS_HI = N > SPLIT
    B = cfg["B"]
    FIN = cfg["FIN"]
    NL = cfg["NL"]
    GH = -(-B // 128)            # graph-halves (1 if B<=128, else 2)
    chunks = _chunks(NSH, CH)
    NCH = len(chunks)
    inv_n = 1.0 / float(N)
    enc_groups = [list(range(GS)), list(range(GS, 2 * GS))]
    pair_groups = [[c, c + GS] for c in range(GS)]

    nc = bacc.Bacc("TRN2", target_bir_lowering=False, debug=False,
                   num_devices=NCORES)

    def din(name, shape, dtype=F32):
        return nc.dram_tensor(name, shape, dtype, kind="ExternalInput")

    xin = din("xin", [FIN, NSH])
    srcs_d = din("srcs", [128, NT * 8], mybir.dt.int16)
    dsts_d = din("dsts", [128, NT])
    batf_d = din("batf", [128, NW])
    io128_d = din("io128", [128, WIN])
    iog_d = din("iog", [128, B])
    idn_d = din("idn", [128, 128])
    ones_d = din("ones", [128, 1])
    trow_d = din("trow", [1, B])

    embw_d = din("emb_w", [FIN, HID])
    embb_d = din("emb_b", [HID, 1])
    lp = []
    for l in range(NL):
        lp.append({k: din(f"{k}_{l}", shp) for k, shp in [
            ("w1", [HID, HID2]),
            ("g1a", [128, 1]), ("g1b", [128, 1]),
            ("be1a", [128, 1]), ("be1b", [128, 1]),
            ("w2a", [128, HID]), ("w2b", [128, HID]),
            ("g2", [HID, 1]), ("be2", [HID, 1]),
            ("epsi", [128, 128]),
        ]})
    hp = {k: din(k, shp) for k, shp in [
        ("pw1a", [128, 128]), ("pw1b", [128, 128]), ("pw1c", [32, 128]),
        ("pb1", [128, 1]), ("pw2", [128, 64]), ("pb2", [64, 1]),
        ("pw3", [64, 1]), ("pb3", [1, 1]),
        ("tw1", [1, 32]), ("tb1", [32, 1]), ("tw2", [32, 32]), ("tb2", [32, 1]),
    ]}

    out_d = nc.dram_tensor("out", [1, B], F32, kind="ExternalOutput")

    # internal DRAM
    xtab_in = nc.dram_tensor("xtab_in", [NSH, HID], F32)
    x_table = nc.dram_tensor("x_table", [N, HID], F32)
    xtab_hi = None
    st_bufs = []
    for l in range(NL):
        st_bufs.append((
            nc.dram_tensor(f"s1i_{l}", [128, 4], F32),
            nc.dram_tensor(f"s1o_{l}", [128, 4], F32),
            nc.dram_tensor(f"s2i_{l}", [128, 2], F32),
            nc.dram_tensor(f"s2o_{l}", [128, 2], F32),
        ))
    GH_ = -(-B // 128)
    pacc_in = nc.dram_tensor("pacc_in", [128, GH_ * 132], F32)
    pacc_out = nc.dram_tensor("pacc_out", [128, GH_ * 132], F32)
    pool_in = nc.dram_tensor("pool_in", [128, B], F32)
    pool_out = nc.dram_tensor("pool_out", [256, B], F32)

    with tile.TileContext(nc) as tc:
        with (
            tc.tile_pool(name="persist", bufs=1) as pp,
            tc.tile_pool(name="wrk", bufs=3) as wk,
            tc.tile_pool(name="gat", bufs=3) as gp,
        ):
            # ---- persistent loads ----
            xT = pp.tile([128, NSH], F32, tag="xT")
            dsts = pp.tile([128, NT], F32, tag="dsts")
            batf = pp.tile([128, NW], F32, tag="batf")
            io128 = pp.tile([128, WIN], F32, tag="io128")
            iog = pp.tile([128, B], F32, tag="iog")
            idn = pp.tile([128, 128], F32, tag="idn")
            onec = pp.tile([128, 1], F32, tag="onec")
            trow = pp.tile([1, B], F32, tag="trow")
            ssum = pp.tile([128, 2 * NCH], F32, tag="ssum")
            ssq = pp.tile([128, 2 * NCH], F32, tag="ssq")
            s2sum = pp.tile([128, NCH], F32, tag="s2sum")
            s2sq = pp.tile([128, NCH], F32, tag="s2sq")

            nc.sync.dma_start(out=dsts[:], in_=dsts_d[:, :])
            nc.sync.dma_start(out=batf[:], in_=batf_d[:, :])
            nc.sync.dma_start(out=io128[:], in_=io128_d[:, :])
            nc.sync.dma_start(out=iog[:], in_=iog_d[:, :])
            nc.sync.dma_start(out=idn[:], in_=idn_d[:, :])
            nc.sync.dma_start(out=onec[:], in_=ones_d[:, :])
            nc.sync.dma_start(out=trow[:], in_=trow_d[:, :])

            embw = pp.tile([FIN, HID], F32, tag="embw")
            embb = pp.tile([HID, 1], F32, tag="embb")
            nc.sync.dma_start(out=embw[:], in_=embw_d[:, :])
            nc.sync.dma_start(out=embb[:], in_=embb_d[:, :])
            lt = []
            for l in range(NL):
                d = {}
                for k, h in lp[l].items():
                    d[k] = pp.tile(list(h.shape), F32, tag=f"{k}_{l}",
                                   name=f"{k}_{l}_t")
                    nc.sync.dma_start(out=d[k][:], in_=h[:, :])
                lt.append(d)
            ht = {}
            for k, h in hp.items():
                ht[k] = pp.tile(list(h.shape), F32, tag=k, name=f"{k}_t")
                nc.sync.dma_start(out=ht[k][:], in_=h[:, :])

            with (
                tc.tile_pool(name="pq", bufs=3, space="PSUM") as pqp,
                tc.tile_pool(name="pmlp", bufs=2, space="PSUM") as pmlp,
            ):
                # ---- embedding: xT = emb_w.T @ xin (+ emb_b) ----
                for ci, (c0, cw) in enumerate(chunks):
                    xc = wk.tile([FIN, CH], F32, tag="xc")
                    nc.sync.dma_start(out=xc[:, :cw], in_=xin[:, c0:c0 + cw])
                    pe = pmlp.tile([128, CH], F32, space="PSUM", tag="pm0")
                    nc.tensor.matmul(out=pe[:, :cw], lhsT=embw[:],
                                     rhs=xc[:, :cw], start=True, stop=True)
                    nc.vector.tensor_scalar_add(out=xT[:, c0:c0 + cw],
                                                in0=pe[:, :cw],
                                                scalar1=embb[:, :1])

                # ---- GIN layers ----
                for l in range(NL):
                    d = lt[l]
                    s1i, s1o, s2i, s2o = st_bufs[l]

                    # 1) rebuild the gather table: x_table <- AllGather(x_loc)
                    for w in range(NW):
                        w0 = w * WIN
                        ww = min(WIN, NSH - w0)
                        tp = pqp.tile([128, 128], F32, space="PSUM", tag="pq")
                        nc.tensor.transpose(out=tp[:ww, :],
                                            in_=xT[:, w0:w0 + ww],
                                            identity=idn[:])
                        ts = wk.tile([128, 128], F32, tag="tts")
                        nc.scalar.copy(out=ts[:ww, :], in_=tp[:ww, :])
                        nc.sync.dma_start(out=xtab_in[w0:w0 + ww, :],
                                          in_=ts[:ww, :])
                    nc.gpsimd.collective_compute(
                        "AllGather", ALU.bypass, replica_groups=enc_groups,
                        ins=[xtab_in.ap().opt()], outs=[x_table.ap().opt()])


                    # phase-3 emitter: mm1 + BN1 stats for one chunk
                    def emit_p3(ci, d=d):
                        c0, cw = chunks[ci]
                        for hf in range(2):
                            pm = pmlp.tile([128, CH], F32, space="PSUM",
                                           tag=f"pm{hf}", name=f"pm_{hf}_{ci}")
                            nc.tensor.matmul(
                                out=pm[:, :cw],
                                lhsT=d["w1"][:, hf * 128:(hf + 1) * 128],
                                rhs=xT[:, c0:c0 + cw], start=True, stop=True)
                            sc1 = wk.tile([128, CH], F32, tag="sc1",
                                          name=f"sc1_{ci}_{hf}")
                            nc.scalar.activation(
                                out=sc1[:, :cw], in_=pm[:, :cw], func=AF.Copy,
                                accum_out=ssum[:, hf * NCH + ci:
                                               hf * NCH + ci + 1])
                            sc2 = wk.tile([128, CH], F32, tag="sc2",
                                          name=f"sc2_{ci}_{hf}")
                            nc.scalar.activation(
                                out=sc2[:, :cw], in_=pm[:, :cw],
                                func=AF.Square,
                                accum_out=ssq[:, hf * NCH + ci:
                                              hf * NCH + ci + 1])

                    next_chunk = 0

                    # 2) aggregate + h = (1+eps)*x + agg   (in place on xT)
                    for gi, (ws, lo_bases, hi_bases, gbase, gtiles) in \
                            enumerate(groups):
                        GTl = sum(T_lo[w] for w in ws)
                        GTh = sum(T_hi[w] for w in ws)
                        gt = None
                        if gtiles > 0:
                            idxw = gp.tile([128, GTmax * 8], mybir.dt.int16,
                                           tag="idxw", bufs=2)
                            nc.sync.dma_start(
                                out=idxw[:, :gtiles * 8],
                                in_=srcs_d[:, gbase * 8:(gbase + gtiles) * 8])
                            gt = gp.tile([128, GTmax * HID], F32, tag="gt",
                                         bufs=2)
                            if GTl > 0:
                                nc.gpsimd.dma_gather(
                                    out_ap=gt[:, :GTl * HID].rearrange(
                                        "p (t d) -> p t d", d=HID),
                                    in_ap=x_table[:, :],
                                    idxs_ap=idxw[:, :GTl * 8],
                                    num_idxs=GTl * 128,
                                    num_idxs_reg=GTl * 128,
                                    elem_size=HID,
                                    single_packet=False)
                            if GTh > 0:
                                nc.gpsimd.dma_gather(
                                    out_ap=gt[:, GTl * HID:gtiles * HID
                                              ].rearrange(
                                        "p (t d) -> p t d", d=HID),
                                    in_ap=x_table[SPLIT:, :],
                                    idxs_ap=idxw[:, GTl * 8:gtiles * 8],
                                    num_idxs=GTh * 128,
                                    num_idxs_reg=GTh * 128,
                                    elem_size=HID,
                                    single_packet=False)
                        for w in ws:
                            Tl, Th = T_lo[w], T_hi[w]
                            Tw = Tl + Th
                            w0 = w * WIN
                            ww = min(WIN, NSH - w0)
                            pag = pqp.tile([128, WIN], F32, space="PSUM",
                                           tag="pq")
                            nc.tensor.matmul(out=pag[:, :ww],
                                             lhsT=d["epsi"][:],
                                             rhs=xT[:, w0:w0 + ww],
                                             start=True, stop=(Tw == 0))
                            if Tw > 0:
                                mt = gp.tile([128, Tmax * WIN], F32, tag="mt",
                                             bufs=2)
                                for si, (sb, Ts) in enumerate(
                                        ((lo_bases[w], Tl), (hi_bases[w], Th))):
                                    if Ts == 0:
                                        continue
                                    moff = 0 if si == 0 else Tl * ww
                                    din_ap = dsts[:, sb:sb + Ts]
                                    in0 = bass.AP(
                                        din_ap.tensor, din_ap.offset,
                                        [din_ap.ap[0], [din_ap.ap[1][0], Ts],
                                         [0, ww]])
                                    io_ap = io128[:, :ww]
                                    in1 = bass.AP(
                                        io_ap.tensor, io_ap.offset,
                                        [io_ap.ap[0], [0, Ts], [1, ww]])
                                    nc.vector.tensor_tensor(
                                        out=mt[:, moff:moff + Ts * ww
                                               ].rearrange(
                                            "p (t w) -> p t w", t=Ts),
                                        in0=in0, in1=in1, op=ALU.is_equal)
                                for t in range(Tw):
                                    grid_col = (lo_bases[w] + t if t < Tl
                                                else hi_bases[w] + (t - Tl))
                                    nc.tensor.matmul(
                                        out=pag[:, :ww],
                                        lhsT=gt[:, (grid_col - gbase) * HID:
                                                (grid_col - gbase + 1) * HID],
                                        rhs=mt[:, t * ww:(t + 1) * ww],
                                        start=False, stop=(t == Tw - 1))
                            nc.vector.tensor_copy(out=xT[:, w0:w0 + ww],
                                                  in_=pag[:, :ww])
                        last_win = min(2 * gi + 1, NW - 1)
                        while (next_chunk < NCH and
                               (min((chunks[next_chunk][0] +
                                     chunks[next_chunk][1] - 1) // WIN,
                                    NW - 1) <= last_win)):
                            emit_p3(next_chunk)
                            next_chunk += 1

                    while next_chunk < NCH:
                        emit_p3(next_chunk)
                        next_chunk += 1

                    st = wk.tile([128, 4], F32, tag="st")
                    for hf in range(2):
                        nc.vector.tensor_reduce(
                            out=st[:, hf:hf + 1],
                            in_=ssum[:, hf * NCH:(hf + 1) * NCH],
                            axis=mybir.AxisListType.X, op=ALU.add)
                        nc.vector.tensor_reduce(
                            out=st[:, 2 + hf:3 + hf],
                            in_=ssq[:, hf * NCH:(hf + 1) * NCH],
                            axis=mybir.AxisListType.X, op=ALU.add)
                    nc.sync.dma_start(out=s1i[:, :], in_=st[:])
                    nc.gpsimd.collective_compute(
                        "AllReduce", ALU.add, replica_groups=enc_groups,
                        ins=[s1i.ap().opt()], outs=[s1o.ap().opt()])
                    sr = wk.tile([128, 4], F32, tag="sr")
                    nc.sync.dma_start(out=sr[:], in_=s1o[:, :])

                    sca, sha = [], []
                    for hf in range(2):
                        g_t = d["g1a"] if hf == 0 else d["g1b"]
                        be_t = d["be1a"] if hf == 0 else d["be1b"]
                        mean = wk.tile([128, 1], F32, tag=f"mean{hf}")
                        nc.vector.tensor_scalar_mul(
                            out=mean[:], in0=sr[:, hf:hf + 1], scalar1=inv_n)
                        var = wk.tile([128, 1], F32, tag=f"var{hf}")
                        # var = sq/n - mean^2 ; then + eps
                        nc.vector.tensor_scalar_mul(
                            out=var[:], in0=sr[:, 2 + hf:3 + hf], scalar1=inv_n)
                        m2 = wk.tile([128, 1], F32, tag=f"m2{hf}")
                        nc.vector.tensor_tensor(out=m2[:], in0=mean[:],
                                                in1=mean[:], op=ALU.mult)
                        nc.vector.tensor_tensor(out=var[:], in0=var[:],
                                                in1=m2[:], op=ALU.subtract)
                        nc.vector.tensor_scalar_add(out=var[:], in0=var[:],
                                                    scalar1=BN_EPS)
                        nc.vector.reciprocal(out=var[:], in_=var[:])
                        inv = wk.tile([128, 1], F32, tag=f"inv{hf}")
                        nc.scalar.activation(out=inv[:], in_=var[:],
                                             func=AF.Sqrt)
                        sc = wk.tile([128, 1], F32, tag=f"sc{hf}")
                        nc.vector.tensor_tensor(out=sc[:], in0=g_t[:],
                                                in1=inv[:], op=ALU.mult)
                        sh = wk.tile([128, 1], F32, tag=f"sh{hf}")
                        nc.vector.tensor_tensor(out=sh[:], in0=mean[:],
                                                in1=sc[:], op=ALU.mult)
                        nc.vector.tensor_tensor(out=sh[:], in0=be_t[:],
                                                in1=sh[:], op=ALU.subtract)
                        sca.append(sc)
                        sha.append(sh)

                    # 4) BN1 apply + relu + matmul2 + BN2 stats (recompute mm1)
                    for ci, (c0, cw) in enumerate(chunks):
                        hrs = []
                        for hf in range(2):
                            pm = pmlp.tile([128, CH], F32, space="PSUM",
                                           tag=f"pm{hf}")
                            nc.tensor.matmul(
                                out=pm[:, :cw],
                                lhsT=d["w1"][:, hf * 128:(hf + 1) * 128],
                                rhs=xT[:, c0:c0 + cw], start=True, stop=True)
                            hr = wk.tile([128, CH], F32, tag=f"hr{hf}")
                            nc.scalar.activation(
                                out=hr[:, :cw], in_=pm[:, :cw], func=AF.Relu,
                                bias=sha[hf][:, :1], scale=sca[hf][:, :1])
                            hrs.append(hr)
                        pm2 = pmlp.tile([128, CH], F32, space="PSUM", tag="pm2",
                                        bufs=1)
                        nc.tensor.matmul(out=pm2[:, :cw], lhsT=d["w2a"][:],
                                         rhs=hrs[0][:, :cw], start=True,
                                         stop=False)
                        nc.tensor.matmul(out=pm2[:, :cw], lhsT=d["w2b"][:],
                                         rhs=hrs[1][:, :cw], start=False,
                                         stop=True)
                        nc.scalar.activation(
                            out=xT[:, c0:c0 + cw], in_=pm2[:, :cw],
                            func=AF.Copy,
                            accum_out=s2sum[:, ci:ci + 1])
                        sc2 = wk.tile([128, CH], F32, tag="sc2")
                        nc.scalar.activation(
                            out=sc2[:, :cw], in_=pm2[:, :cw], func=AF.Square,
                            accum_out=s2sq[:, ci:ci + 1])

                    st2 = wk.tile([128, 2], F32, tag="st2")
                    nc.vector.tensor_reduce(out=st2[:, 0:1], in_=s2sum[:, :],
                                            axis=mybir.AxisListType.X,
                                            op=ALU.add)
                    nc.vector.tensor_reduce(out=st2[:, 1:2], in_=s2sq[:, :],
                                            axis=mybir.AxisListType.X,
                                            op=ALU.add)
                    nc.sync.dma_start(out=s2i[:, :], in_=st2[:])
                    nc.gpsimd.collective_compute(
                        "AllReduce", ALU.add, replica_groups=enc_groups,
                        ins=[s2i.ap().opt()], outs=[s2o.ap().opt()])
                    sr2 = wk.tile([128, 2], F32, tag="sr2")
                    nc.sync.dma_start(out=sr2[:], in_=s2o[:, :])

                    mean = wk.tile([128, 1], F32, tag="mean2")
                    nc.vector.tensor_scalar_mul(out=mean[:], in0=sr2[:, 0:1],
                                                scalar1=inv_n)
                    var = wk.tile([128, 1], F32, tag="var2")
                    nc.vector.tensor_scalar_mul(out=var[:], in0=sr2[:, 1:2],
                                                scalar1=inv_n)
                    m2 = wk.tile([128, 1], F32, tag="m22")
                    nc.vector.tensor_tensor(out=m2[:], in0=mean[:], in1=mean[:],
                                            op=ALU.mult)
                    nc.vector.tensor_tensor(out=var[:], in0=var[:], in1=m2[:],
                                            op=ALU.subtract)
                    nc.vector.tensor_scalar_add(out=var[:], in0=var[:],
                                                scalar1=BN_EPS)
                    nc.vector.reciprocal(out=var[:], in_=var[:])
                    inv = wk.tile([128, 1], F32, tag="inv2")
                    nc.scalar.activation(out=inv[:], in_=var[:], func=AF.Sqrt)
                    sc2t = wk.tile([128, 1], F32, tag="sc2t")
                    nc.vector.tensor_tensor(out=sc2t[:], in0=d["g2"][:],
                                            in1=inv[:], op=ALU.mult)
                    sh2t = wk.tile([128, 1], F32, tag="sh2t")
                    nc.vector.tensor_tensor(out=sh2t[:], in0=mean[:],
                                            in1=sc2t[:], op=ALU.mult)
                    nc.vector.tensor_tensor(out=sh2t[:], in0=d["be2"][:],
                                            in1=sh2t[:], op=ALU.subtract)

                    for ci, (c0, cw) in enumerate(chunks):
                        nc.scalar.activation(out=xT[:, c0:c0 + cw],
                                             in_=xT[:, c0:c0 + cw],
                                             func=AF.Relu, bias=sh2t[:, :1],
                                             scale=sc2t[:, :1])

            # ---- pooling + head (fresh PSUM pools) ----
            with (
                tc.tile_pool(name="ppool", bufs=1, space="PSUM") as ppl,
                tc.tile_pool(name="ptr", bufs=2, space="PSUM") as ptr,
                tc.tile_pool(name="phd", bufs=2, space="PSUM") as phd,
            ):
                # tags: plg0/plg1 (1 bank each), ptp (2), ph (2) => 6 banks
                plg = [ppl.tile([128, 132], F32, space="PSUM", tag=f"plg{g}",
                                name=f"plg{g}")
                       for g in range(GH)]
                for w in range(NW):
                    w0 = w * WIN
                    ww = min(WIN, NSH - w0)
                    tp = ptr.tile([128, 128], F32, space="PSUM", tag="ptp")
                    nc.tensor.transpose(out=tp[:ww, :], in_=xT[:, w0:w0 + ww],
                                        identity=idn[:])
                    xw = wk.tile([128, 132], F32, tag="xw")
                    if ww < 128:
                        nc.vector.memset(xw[:], 0.0)
                    nc.scalar.copy(out=xw[:ww, :128], in_=tp[:ww, :])
                    nc.vector.tensor_copy(out=xw[:, 128:129], in_=onec[:, :1])
                    bt = wk.tile([128, B], F32, tag="bt")
                    b_ap = batf[:, w:w + 1]
                    nc.vector.tensor_tensor(
                        out=bt[:, :B],
                        in0=bass.AP(b_ap.tensor, b_ap.offset,
                                    [b_ap.ap[0], [0, B]]),
                        in1=iog[:, :B], op=ALU.is_equal)
                    for g in range(GH):
                        gw = min(128, B - g * 128)
                        nc.tensor.matmul(out=plg[g][:gw, 0:129],
                                         lhsT=bt[:, g * 128:g * 128 + gw],
                                         rhs=xw[:, :129], start=(w == 0),
                                         stop=(w == NW - 1))

                # combine shard-partial pooled sums/counts across the group
                pacc = wk.tile([128, GH * 132], F32, tag="pacc")
                nc.vector.memset(pacc[:], 0.0)
                for g in range(GH):
                    gw = min(128, B - g * 128)
                    nc.scalar.copy(out=pacc[:gw, g * 132:g * 132 + 129],
                                   in_=plg[g][:gw, :129])
                nc.sync.dma_start(out=pacc_in[:, :], in_=pacc[:])
                nc.gpsimd.collective_compute(
                    "AllReduce", ALU.add, replica_groups=enc_groups,
                    ins=[pacc_in.ap().opt()], outs=[pacc_out.ap().opt()])
                pr = wk.tile([128, GH * 132], F32, tag="pr")
                nc.sync.dma_start(out=pr[:], in_=pacc_out[:, :])

                embT = wk.tile([128, B], F32, tag="embT")
                for g in range(GH):
                    gw = min(128, B - g * 128)
                    invc = wk.tile([128, 1], F32, tag="invc")
                    nc.vector.tensor_scalar_max(
                        out=invc[:gw, :],
                        in0=pr[:gw, g * 132 + 128:g * 132 + 129],
                        scalar1=1.0)
                    nc.vector.reciprocal(out=invc[:gw, :], in_=invc[:gw, :])
                    pgs = wk.tile([128, 128], F32, tag="pgs")
                    nc.scalar.activation(out=pgs[:gw, :],
                                         in_=pr[:gw, g * 132:g * 132 + 128],
                                         func=AF.Copy, scale=invc[:gw, :1])
                    tpp = ptr.tile([128, 128], F32, space="PSUM", tag="ptp")
                    nc.tensor.transpose(out=tpp[:, :gw], in_=pgs[:gw, :],
                                        identity=idn[:gw, :gw])
                    nc.scalar.copy(out=embT[:, g * 128:g * 128 + gw],
                                   in_=tpp[:, :gw])

                nc.sync.dma_start(out=pool_in[:, :], in_=embT[:])
                nc.gpsimd.collective_compute(
                    "AllGather", ALU.bypass, replica_groups=pair_groups,
                    ins=[pool_in.ap().opt()], outs=[pool_out.ap().opt()])
                demb = wk.tile([128, B], F32, tag="demb")
                semb = wk.tile([128, B], F32, tag="semb")
                nc.sync.dma_start(out=demb[:], in_=pool_out[0:128, :])
                nc.sync.dma_start(out=semb[:], in_=pool_out[128:256, :])

                # temperature MLP
                ptm = phd.tile([32, B], F32, space="PSUM", tag="ph")
                nc.tensor.matmul(out=ptm[:], lhsT=ht["tw1"][:], rhs=trow[:],
                                 start=True, stop=True)
                t1 = wk.tile([32, B], F32, tag="t1")
                nc.scalar.activation(out=t1[:], in_=ptm[:], func=AF.Relu,
                                     bias=ht["tb1"][:, :1])
                ptm2 = phd.tile([32, B], F32, space="PSUM", tag="ph")
                nc.tensor.matmul(out=ptm2[:], lhsT=ht["tw2"][:], rhs=t1[:],
                                 start=True, stop=True)
                t2 = wk.tile([32, B], F32, tag="t2")
                nc.vector.tensor_scalar_add(out=t2[:], in0=ptm2[:],
                                            scalar1=ht["tb2"][:, :1])

                # prediction head
                ph1 = phd.tile([128, B], F32, space="PSUM", tag="ph")
                nc.tensor.matmul(out=ph1[:], lhsT=ht["pw1a"][:], rhs=demb[:],
                                 start=True, stop=False)
                nc.tensor.matmul(out=ph1[:], lhsT=ht["pw1b"][:], rhs=semb[:],
                                 start=False, stop=False)
                nc.tensor.matmul(out=ph1[:], lhsT=ht["pw1c"][:], rhs=t2[:],
                                 start=False, stop=True)
                h1s = wk.tile([128, B], F32, tag="h1s")
                nc.scalar.activation(out=h1s[:], in_=ph1[:], func=AF.Relu,
                                     bias=ht["pb1"][:, :1])
                ph2 = phd.tile([64, B], F32, space="PSUM", tag="ph")
                nc.tensor.matmul(out=ph2[:], lhsT=ht["pw2"][:], rhs=h1s[:],
                                 start=True, stop=True)
                h2s = wk.tile([64, B], F32, tag="h2s")
                nc.scalar.activation(out=h2s[:], in_=ph2[:], func=AF.Relu,
                                     bias=ht["pb2"][:, :1])
                ph3 = phd.tile([1, B], F32, space="PSUM", tag="ph")
                nc.tensor.matmul(out=ph3[:], lhsT=ht["pw3"][:], rhs=h2s[:],
                                 start=True, stop=True)
                oT = wk.tile([1, B], F32, tag="oT")
                nc.vector.tensor_scalar_add(out=oT[:], in0=ph3[:],
                                            scalar1=ht["pb3"][:, :1])
                nc.sync.dma_start(out=out_d[:, :], in_=oT[:])

    nc.compile()
    return nc


# ---------------------------------------------------------------------------
# Input packing
# ---------------------------------------------------------------------------

def _enc_param_maps(enc, NL):
    """Per-encoder named parameter arrays for the device program."""
    out = {
        "emb_w": np.asarray(enc["emb_w"], np.float32),
        "emb_b": np.asarray(enc["emb_b"], np.float32).reshape(HID, 1),
    }
    for l in range(NL):
        p = enc["layers"][l]
        w1 = np.asarray(p["w1"], np.float32)
        g1 = np.asarray(p["g1"], np.float32)
        be1 = np.asarray(p["be1"], np.float32)
        w2 = np.asarray(p["w2"], np.float32)
        eps = float(np.asarray(p["eps"]))
        out[f"w1_{l}"] = w1
        out[f"g1a_{l}"] = g1[:128].reshape(128, 1)
        out[f"g1b_{l}"] = g1[128:].reshape(128, 1)
        out[f"be1a_{l}"] = be1[:128].reshape(128, 1)
        out[f"be1b_{l}"] = be1[128:].reshape(128, 1)
        out[f"w2a_{l}"] = w2[:128]
        out[f"w2b_{l}"] = w2[128:]
        out[f"g2_{l}"] = np.asarray(p["g2"], np.float32).reshape(HID, 1)
        out[f"be2_{l}"] = np.asarray(p["be2"], np.float32).reshape(HID, 1)
        out[f"epsi_{l}"] = ((1.0 + eps) * np.eye(128)).astype(np.float32)
    return out


def make_in_maps(inputs, NL=4):
    drug_x = np.asarray(inputs["drug_x"], np.float32)
    solv_x = np.asarray(inputs["solvent_x"], np.float32)
    N, FIN = drug_x.shape
    assert N % GS == 0
    NSH = N // GS
    NW = -(-NSH // WIN)
    params = inputs["params"]
    temperature = np.asarray(inputs["temperature"], np.float32)
    B = temperature.shape[0]

    SPLIT = min(32768, N)
    if "split_override" in inputs:
        SPLIT = int(inputs["split_override"])
    enc_data = []
    counts_lo, counts_hi = [], []
    for key_x, key_e, key_b, key_p in (
        ("drug_x", "drug_edge_index", "drug_batch", "drug"),
        ("solvent_x", "solvent_edge_index", "solvent_batch", "solvent"),
    ):
        x = np.asarray(inputs[key_x], np.float32)
        shards = _shard_edges(np.asarray(inputs[key_e]), N, NSH)
        batch = np.asarray(inputs[key_b], np.int64)
        enc_data.append((x, shards, batch, params[key_p]))
        for ss, dd in shards:
            clo, chi = _window_stream_counts(ss, dd, NSH, SPLIT)
            counts_lo.append(clo)
            counts_hi.append(chi)
    T_lo = [int(max(-(-counts_lo[i][w] // 128) for i in range(len(counts_lo))))
            for w in range(NW)]
    T_hi = [int(max(-(-counts_hi[i][w] // 128) for i in range(len(counts_hi))))
            for w in range(NW)]

    io128 = np.tile(np.arange(WIN, dtype=np.float32)[None, :], (128, 1))
    iog = np.tile(np.arange(B, dtype=np.float32)[None, :], (128, 1))
    idn = np.eye(128, dtype=np.float32)
    ones = np.ones((128, 1), np.float32)
    trow = np.ascontiguousarray(temperature.reshape(1, B))

    hp = params["pred"]
    tp = params["temp"]
    pw1 = np.asarray(hp["w1"], np.float32)
    head = {
        "pw1a": pw1[:128], "pw1b": pw1[128:256], "pw1c": pw1[256:288],
        "pb1": np.asarray(hp["b1"], np.float32).reshape(128, 1),
        "pw2": np.asarray(hp["w2"], np.float32),
        "pb2": np.asarray(hp["b2"], np.float32).reshape(64, 1),
        "pw3": np.asarray(hp["w3"], np.float32),
        "pb3": np.asarray(hp["b3"], np.float32).reshape(1, 1),
        "tw1": np.asarray(tp["w1"], np.float32),
        "tb1": np.asarray(tp["b1"], np.float32).reshape(32, 1),
        "tw2": np.asarray(tp["w2"], np.float32),
        "tb2": np.asarray(tp["b2"], np.float32).reshape(32, 1),
    }

    in_maps = []
    for c in range(NCORES):
        e = c // GS
        s = c % GS
        x, shards, batch, enc_p = enc_data[e]
        src_a, dst_a = _build_edge_arrays(shards[s][0], shards[s][1], NSH,
                                          T_lo, T_hi, SPLIT)
        m = {
            "xin": np.ascontiguousarray(x[s * NSH:(s + 1) * NSH].T),
            "srcs": src_a,
            "dsts": dst_a,
            "batf": _batch_array(batch, NSH, s),
            "io128": io128, "iog": iog, "idn": idn, "ones": ones,
            "trow": trow,
        }
        m.update(_enc_param_maps(enc_p, NL))
        m.update(head)
        in_maps.append(m)

    cfg = {"NSH": NSH, "B": B, "FIN": FIN, "NL": NL,
           "T_lo": T_lo, "T_hi": T_hi, "SPLIT": SPLIT}
    return cfg, in_maps


_PROGRAM_CACHE = {}


def _get_program(cfg):
    key = (cfg["NSH"], cfg["B"], cfg["FIN"], cfg["NL"],
           tuple(cfg["T_lo"]), tuple(cfg["T_hi"]), cfg["SPLIT"])
    if key not in _PROGRAM_CACHE:
        _PROGRAM_CACHE[key] = build_program(cfg)
    return _PROGRAM_CACHE[key]


def kernel(**inputs) -> np.ndarray:
    from concourse.bass_utils import run_bass_kernel_spmd
    cfg, in_maps = make_in_maps(inputs)
    nc = _get_program(cfg)
    res = run_bass_kernel_spmd(nc, in_maps, core_ids=list(range(NCORES)))
    out = np.asarray(res.results[0]["out"], np.float32)
    return out.reshape(-1, 1)
